# revision 63
# baseline (speedup 1.0000x reference)
"""Trainium2 Bass kernel for multi-head causal attention (v3, bf16+fp8).

Problem (hardcoded): x [2, 2048, 1024] fp32, w_qkv [1024, 3072], w_out
[1024, 1024].
  qkv = x @ w_qkv; per-head causal softmax attention (16 heads, d=64);
  out = attn_out @ w_out.

Sharding: 8 cores = (2 batches) x (4 head-groups of 4 heads).
Each core computes, for its batch b and heads 4g..4g+3 (2 pairs of 2 heads):
  - Q^T, K^T [256, 2048] and V [2048, 256] from x[b]^T (host-pretransposed,
    bf16) via PE, pipelined as filler work between attention groups
  - causal attention on-chip in S^T layout (S in bf16 — fp8 QK noise breaks
    tolerance); diagonal tiles are column-sliced so fully-masked columns are
    never computed, the remaining 128-col triangle zeroed post-exp by a
    gpsimd affine_select; rowsum via a ones-column in the AV stationary
  - AV for q-chunks j>=1 runs in fp8e4 DoubleRow (2 k-tiles per matmul at
    0.5 cyc/col; exp writes p2t as fp8 with a -4 logit shift so the
    numerator can't overflow e4m3's 448 max — the shift cancels in the
    normalize).  Chunk 0 keeps the bf16 path: with <512 keys the fp8 P/V
    quantization noise doesn't average out.
  - per-head group ordering (h2-outer): each head's AV bank drains (at-copy
    on ACT, reciprocal on DVE, DRAM-hop broadcast DMA) while the other
    head's attention still runs; normalization mul on DVE once both
    broadcast halves land
  - out-projection + drains pipelined against the next block; attention
    tiles parity-alternate between reps so a benched rep's projections
    overlap the previous rep's tail
Host gathers: y[b] = sum_g y_part[4b+g] (partials stored bf16).
"""
import numpy as np

import concourse.bass as bass
from concourse import bacc
import concourse.mybir as mybir
import concourse.tile as tile

F32 = mybir.dt.float32
F32R = mybir.dt.float32r
BF16 = mybir.dt.bfloat16
FP8 = mybir.dt.float8e4
AF = mybir.ActivationFunctionType
DR = mybir.MatmulPerfMode.DoubleRow

B, T, C = 2, 2048, 1024
H_TOT, D = 16, 64
HL = 4             # heads per core
DL = HL * D        # 256 local channels
NJ = 4             # q-chunks of 512
NKT = 16           # k-tiles of 128
NCT = 8            # c-tiles of 128 (contraction over C)
SM_SCALE = 1.0 / np.sqrt(D)
NEG = -30000.0

_CACHE = {}

# variant used by kernel() and by test.py's default bench:
#   fp8   - AV via fp8e4 DoubleRow for chunks j>=1 (chunk 0 stays bf16)
#   dexp1 - single-span exp on diagonal groups
#   xrep  - parity-alternated attention tiles (cross-rep overlap)
#   atact - drain PSUM->SBUF copies on ACT (frees the DVE release path)
#   h2o   - per-head group ordering with early per-head drains
#   pend2 - AV emission lags S/exp by 2 groups
#   fdefer - defer late filler copies past the drain (DVE queue priority)
DEFAULT_SKIP = ("fp8", "dexp1", "xrep", "atact", "h2o", "pend2", "fdefer")


def build_nc(reps=1, skip=()):
    nc = bacc.Bacc("TRN2", target_bir_lowering=False)
    xt = nc.dram_tensor("xt", [C, T], BF16, kind="ExternalInput")
    wqkv = nc.dram_tensor("wqkv", [C, 3 * DL], BF16, kind="ExternalInput")
    wo = nc.dram_tensor("wo", [DL, C], BF16, kind="ExternalInput")
    msk = nc.dram_tensor("msk", [128, 256], BF16, kind="ExternalInput")
    y = nc.dram_tensor("y", [T, C], BF16, kind="ExternalOutput")
    if "rcout" in skip:
        rcdbg = nc.dram_tensor("rcdbg", [16, 512], F32, kind="ExternalOutput")

    with tile.TileContext(nc) as tc:
      for _rep in range(reps):
        with tc.tile_pool(name="persist", bufs=1) as persist, \
             tc.tile_pool(name="dram", bufs=1, space="DRAM") as drampool, \
             tc.tile_pool(name="rcp", bufs=6) as rcp, \
             tc.tile_pool(name="bcp", bufs=3) as bcp, \
             tc.tile_pool(name="pexp", bufs=6) as pexp, \
             tc.tile_pool(name="ysbp", bufs=4) as ysbp, \
             tc.tile_pool(name="s2p", bufs=2, space="PSUM") as s2p, \
             tc.tile_pool(name="otp", bufs=1, space="PSUM") as otp, \
             tc.tile_pool(name="ppp", bufs=2, space="PSUM") as ppp:
            use8 = "fp8" in skip
            # cross-rep parity: alternate the attention tiles between reps
            # so rep N+1's projections aren't WAR-blocked on rep N's last
            # S/AV reads (benched steady-state overlaps rep tails)
            par = _rep % 2 if "xrep" in skip else 0
            qk_tiles = [persist.tile([128, T], BF16, tag=f"qk{m}_{par}",
                                     name=f"qk{m}_{par}") for m in range(4)]
            qt_t, kt_t = qk_tiles[0:2], qk_tiles[2:4]
            # S stays bf16 (fp8 QK noise breaks tolerance); AV for chunks
            # j>=1 uses fp8 DoubleRow (chunk 0 has too few keys to average
            # out fp8 P/V noise, so it keeps the bf16 path and only needs
            # bf16 V for k-tiles 0..3).
            nvsb = 4 if use8 else NKT
            v_sb = [persist.tile([128, HL, D + 1], BF16, tag=f"v{t}_{par}",
                                 name=f"v{t}_{par}") for t in range(nvsb)]
            if use8:
                # vp8 per k-tile pair [128p, 2 ktile, HL, 128] (col D = ones
                # rowsum, col D+1 pad: stationary 66 wide; per-head width
                # padded to 128 so the DoubleRow stationary's k-subtile
                # stride is 512B, an ISA restriction)
                vp8 = [persist.tile([128, 2, HL, 128], FP8,
                                    tag=f"vp{t}_{par}",
                                    name=f"vp{t}_{par}")
                       for t in range(NKT // 2)]
                ebias = persist.tile([128, 1], F32, tag="eb", name="eb")
                nc.vector.memset(ebias[:], -4.0)
            at_t = [persist.tile([128, T], BF16, tag=f"at{p}_{par}",
                                 name=f"at{p}_{par}") for p in range(2)]
            xt_sb = [persist.tile([128, T], BF16, tag=f"xt{c}", name=f"xt{c}")
                     for c in range(NCT)]
            wqkv_sb = [persist.tile([128, 3 * DL], BF16, tag=f"wq{c}",
                                    name=f"wq{c}") for c in range(NCT)]
            wo_sb = [persist.tile([128, C], BF16, tag=f"wo{i}", name=f"wo{i}")
                     for i in range(2)]
            msk_sb = persist.tile([128, 256], BF16, tag="msk", name="msk")
            otd = 66 if use8 else 65
            ot_ps = [otp.tile([otd, 512], F32, tag=f"ot{h2}", name=f"ot{h2}")
                     for h2 in range(2)]
            if "rcout" in skip:
                rc_dram = rcdbg[:, :]
            else:
                rc_dram = drampool.tile([16, 512], F32)

            if "probe" in skip:
                pr = persist.tile([128, 2, 512], FP8, tag="pr8", name="pr8")
                pr2 = persist.tile([128, 2, 65], FP8, tag="pr8b", name="pr8b")
                prq = persist.tile([128, 2, 512], FP8, tag="pr8c", name="pr8c")
                prps = ppp.tile([128, 512], F32, tag="pp", name="pp")
                nc.tensor.matmul(prps[:], wqkv_sb[0][:, 0:128],
                                 xt_sb[0][:, 0:512], start=True, stop=True)
                # (a) DVE f32->fp8 cast from PSUM
                nc.vector.tensor_copy(pr[:, 0, :], prps[:])
                # (b) ACT exp with fp8 out
                nc.scalar.activation(pr[:, 1, :], prps[:], AF.Exp, scale=0.01)
                # (c) affine_select on fp8
                nc.gpsimd.affine_select(
                    out=pr[:, 0, 0:128], in_=pr[:, 0, 0:128],
                    compare_op=mybir.AluOpType.is_ge, fill=0.0, base=0,
                    pattern=[[1, 128]], channel_multiplier=-1)
                # (e) fp8 memset
                nc.vector.memset(pr2[:, :, 64:65], 1.0)
                nc.vector.tensor_copy(pr2[:, 0, 0:64], prps[:, 0:64])
                nc.vector.tensor_copy(prq[:, :, 0:256],
                                      prps[:].rearrange("p (k q) -> p k q",
                                                        k=2))
                ps8 = ppp.tile([128, 512], F32, tag="pp", name="pp")
                if "pa" in skip:   # AV-style DoubleRow, M=65 (odd)
                    nc.tensor.matmul(ps8[0:65, :], pr2[:, :, :], pr[:, :, :],
                                     start=True, stop=True, perf_mode=DR)
                if "pb" in skip:   # AV-style DoubleRow, M=64
                    nc.tensor.matmul(ps8[0:64, :], pr2[:, :, 0:64],
                                     pr[:, :, :], start=True, stop=True,
                                     perf_mode=DR)
                if "pc" in skip:   # AV-style DoubleRow, M=66 via pr tile
                    nc.tensor.matmul(ps8[0:66, :], pr[:, :, 0:66],
                                     pr[:, :, :], start=True, stop=True,
                                     perf_mode=DR)
                if "pd" in skip:   # S-style 32-row DoubleRow at base 32
                    nc.tensor.matmul(ps8[:, 0:512], prq[32:64, :, 0:128],
                                     prq[32:64, :, :],
                                     start=True, stop=True, perf_mode=DR)
                if "pe2" in skip:  # mixed accumulation group DR + plain fp8
                    nc.tensor.matmul(ps8[0:66, :], pr[:, :, 0:66],
                                     pr[:, :, :], start=True, stop=False,
                                     perf_mode=DR)
                    nc.tensor.matmul(ps8[0:66, 128:512], pr[:, 0, 0:66],
                                     pr[:, 0, 128:512], start=False,
                                     stop=True)
                nc.vector.tensor_copy(at_t[0][:, 0:512], ps8[:])

            # ---- loads ----
            # xt column-split: chunk-0 columns first so projections for j=0
            # can start before the bulk of x arrives.
            for c in range(NCT):
                eng = nc.sync if c % 2 == 0 else nc.scalar
                eng.dma_start(out=xt_sb[c][:, 0:512],
                              in_=xt[128 * c:128 * (c + 1), 0:512])
                (nc.scalar if c % 2 == 0 else nc.sync).dma_start(
                    out=wqkv_sb[c][:], in_=wqkv[128 * c:128 * (c + 1), :])
            for c in range(NCT):
                (nc.sync if c % 2 == 0 else nc.scalar).dma_start(
                    out=xt_sb[c][:, 512:T],
                    in_=xt[128 * c:128 * (c + 1), 512:T])
            for i in range(2):
                nc.sync.dma_start(out=wo_sb[i][:],
                                  in_=wo[128 * i:128 * (i + 1), :])
            nc.scalar.dma_start(out=msk_sb[:], in_=msk[:, :])
            sum0 = "sum0" in skip
            vco = 1 if sum0 else 0     # v data column offset in stationaries
            if use8:
                for tp in range(NKT // 2):
                    if sum0:
                        nc.vector.memset(vp8[tp][:, :, :, 0:1], 1.0)
                        nc.vector.memset(vp8[tp][:, :, :, 65:66], 1.0)
                    else:
                        nc.vector.memset(vp8[tp][:, :, :, D:D + 2], 1.0)
            for t in range(nvsb):
                if sum0:
                    nc.vector.memset(v_sb[t][:, :, 0:1], 1.0)
                else:
                    nc.vector.memset(v_sb[t][:, :, D:D + 1], 1.0)

            def qk_chunk(m, j, defer=False):
                ps = ppp.tile([128, 512], F32, tag="pp", name="pp")
                for c in range(NCT):
                    nc.tensor.matmul(
                        ps[:],
                        wqkv_sb[c][:, 128 * m:128 * (m + 1)],
                        xt_sb[c][:, 512 * j:512 * (j + 1)],
                        start=(c == 0), stop=(c == NCT - 1))

                def copy():
                    nc.vector.tensor_copy(
                        qk_tiles[m][:, 512 * j:512 * (j + 1)], ps[:])
                if defer:
                    return copy
                copy()

            def v_tile(t, defer=False):
                ps = ppp.tile([128, 512], F32, tag="pp", name="pp")
                for c in range(NCT):
                    nc.tensor.matmul(
                        ps[:, 0:DL],
                        xt_sb[c][:, 128 * t:128 * (t + 1)],
                        wqkv_sb[c][:, 2 * DL:3 * DL],
                        start=(c == 0), stop=(c == NCT - 1))

                def copy():
                    if use8:
                        nc.vector.tensor_copy(
                            vp8[t // 2][:, t % 2, :, vco:vco + D],
                            ps[:, 0:DL].rearrange("p (h d) -> p h d", h=HL))
                        if t < nvsb:
                            nc.vector.tensor_copy(
                                v_sb[t][:, :, vco:vco + D],
                                ps[:, 0:DL].rearrange("p (h d) -> p h d",
                                                      h=HL))
                    else:
                        nc.vector.tensor_copy(
                            v_sb[t][:, :, vco:vco + D],
                            ps[:, 0:DL].rearrange("p (h d) -> p h d", h=HL))
                if defer:
                    return copy
                copy()

            def s_group(pair, j, ktg, h2):
                """Emit S^T matmuls (+pre-exp causal mask) for one s2 group
                (2 k-tiles); returns the s2 tile and exp metadata."""
                s2 = s2p.tile([128, 1024], F32, tag="s", name="s")
                diag = (ktg >= 2 * j)
                segs = []
                for kk in range(2):
                    ktt = 2 * ktg + kk
                    col0 = 512 * kk
                    q0 = 0 if not diag else 128 * (ktt - 4 * j)
                    qt, kt = qt_t[pair], kt_t[pair]
                    base = 64 * h2
                    nc.tensor.matmul(
                        s2[:, col0 + q0:col0 + 512],
                        kt[base:base + 64, 128 * ktt:128 * (ktt + 1)],
                        qt[base:base + 64, 512 * j + q0:512 * (j + 1)],
                        start=True, stop=True)
                    segs.append((col0 + q0, 512 - q0))
                return s2, diag, segs

            def exp_group(s2, diag, segs, p8):
                p2t = pexp.tile([128, 1024], FP8 if p8 else BF16,
                                tag="p8" if p8 else "p",
                                name="p8" if p8 else "p")
                if "noexp" in skip:
                    return p2t
                # fp8e4 has no inf and max 448: shift logits down so the
                # softmax numerator never overflows (cancels in normalize)
                eb = ebias[:, :] if p8 else 0.0
                nrep = 2 if "2xexp" in skip else 1
                if not diag:
                    for _ in range(nrep):
                        nc.scalar.activation(p2t[:], s2[:], AF.Exp,
                                             scale=float(SM_SCALE), bias=eb)
                elif "dexp1" in skip:
                    # single span including the dead gap between segments
                    col_lo = segs[0][0]
                    nc.scalar.activation(p2t[:, col_lo:1024],
                                         s2[:, col_lo:1024], AF.Exp,
                                         scale=float(SM_SCALE), bias=eb)
                else:
                    for col0, w in segs:
                        for _ in range(nrep):
                            nc.scalar.activation(p2t[:, col0:col0 + w],
                                                 s2[:, col0:col0 + w], AF.Exp,
                                                 scale=float(SM_SCALE),
                                                 bias=eb)
                return p2t

            def av_group(pair, j, ktg, h2, p2t, diag, segs):
                """ot bank group: start=True only on the block's first matmul
                (clears the bank), stop=True only on its last (r=3 part A)."""
                h = 2 * pair + h2
                ot = ot_ps[h2]
                p8 = use8 and j >= 1
                if p8 and not diag:
                    # one DoubleRow matmul covers both k-tiles of the group
                    nc.tensor.matmul(
                        ot[:, 0:512], vp8[ktg][:, :, h, 0:D + 2],
                        p2t[:].rearrange("p (k q) -> p k q", k=2),
                        start=(ktg == 0), stop=False, perf_mode=DR)
                    return
                for kk in range(2):
                    ktt = 2 * ktg + kk
                    col0 = 512 * kk
                    if p8:
                        vv = vp8[ktg][:, kk, h, 0:D + 2]
                        ot = ot_ps[h2]
                    else:
                        vv = v_sb[ktt][:, h, 0:D + 1]
                        ot = ot_ps[h2][0:65]
                    if not diag:
                        nc.tensor.matmul(ot[:, 0:512], vv,
                                         p2t[:, col0:col0 + 512],
                                         start=(ktt == 0), stop=False)
                        if "2xav" in skip:
                            nc.tensor.matmul(ot[:, 0:512], vv,
                                             p2t[:, col0:col0 + 512],
                                             start=False, stop=False)
                    else:
                        r = ktt - 4 * j
                        q0 = 128 * r
                        if "nomask" not in skip:
                            nc.gpsimd.affine_select(
                                out=p2t[:, col0 + q0:col0 + q0 + 128],
                                in_=p2t[:, col0 + q0:col0 + q0 + 128],
                                compare_op=mybir.AluOpType.is_ge,
                                fill=0.0, base=0,
                                pattern=[[1, 128]],
                                channel_multiplier=-1)
                        first = (j == 0 and r == 0)
                        nc.tensor.matmul(ot[:, q0:512], vv,
                                         p2t[:, col0 + q0:col0 + 512],
                                         start=first, stop=(r == 3))

            def att_block(pair, j, fillers=()):
                """Software-pipelined S -> exp -> AV over all groups.
                `fillers` are independent emission closures (projections for
                the next chunk) slotted between groups to keep PE fed while
                the exp chain runs."""
                fillers = list(fillers)
                deferred = []
                fdef = "fdefer" in skip
                lag = 3 if "pend3" in skip else (2 if "pend2" in skip else 1)
                if "h2o" in skip:
                    # h2-outer: finish one head's groups (and its drain)
                    # before the other's, so each ot bank is released and
                    # normalized mid-block instead of both at block end
                    bc = bcp.tile([128, 512], F32, tag="bc", name="bc")
                    ng = 2 * (j + 1)
                    for h2 in range(2):
                        pend = []
                        for ktg in range(ng):
                            s2, diag, segs = s_group(pair, j, ktg, h2)
                            p2t = exp_group(s2, diag, segs, use8 and j >= 1)
                            pend.append((ktg, h2, p2t, diag, segs))
                            if fillers:
                                late = fdef and (h2 == 1 and ktg >= ng - 3)
                                d = fillers.pop(0)(late)
                                if d is not None:
                                    deferred.append(d)
                            if len(pend) > lag:
                                g = pend.pop(0)
                                av_group(pair, j, g[0], g[1], g[2], g[3],
                                         g[4])
                        for g in pend:
                            av_group(pair, j, g[0], g[1], g[2], g[3], g[4])
                        drain_h2(pair, j, h2, bc)
                    drain_mul(pair, j, bc)
                    for f in fillers:
                        d = f(fdef)
                        if d is not None:
                            deferred.append(d)
                    return deferred
                glist = [(ktg, h2) for ktg in range(2 * (j + 1))
                         for h2 in range(2)]
                pend = []   # (ktg, h2, p2t, diag, segs) awaiting AV emission
                for i, (ktg, h2) in enumerate(glist):
                    s2, diag, segs = s_group(pair, j, ktg, h2)
                    p2t = exp_group(s2, diag, segs, use8 and j >= 1)
                    pend.append((ktg, h2, p2t, diag, segs))
                    if fillers:
                        late = fdef and i >= len(glist) - 3
                        d = fillers.pop(0)(late)
                        if d is not None:
                            deferred.append(d)
                    if i >= lag:
                        g = pend.pop(0)
                        av_group(pair, j, g[0], g[1], g[2], g[3], g[4])
                for g in pend:
                    av_group(pair, j, g[0], g[1], g[2], g[3], g[4])
                for f in fillers:
                    d = f(fdef)
                    if d is not None:
                        deferred.append(d)
                return deferred

            def drain_h2(pair, j, h2, bc):
                """Per-head drain: at-copy + recip + broadcast into bc half."""
                cp = (nc.scalar.copy if "atact" in skip
                      else nc.vector.tensor_copy)
                cp(at_t[pair][64 * h2:64 * h2 + 64,
                              512 * j:512 * (j + 1)],
                   ot_ps[h2][0:64, :])
                rc = rcp.tile([65, 512], F32, tag="rc", name="rc")
                nc.vector.reciprocal(out=rc[64:65, :],
                                     in_=ot_ps[h2][64:65, :])
                idx = 4 * j + 2 * pair + h2
                dq = nc.gpsimd if "rcpool" in skip else nc.sync
                dq.dma_start(out=rc_dram[idx:idx + 1, :], in_=rc[64:65, :])
                seg = rc_dram[idx:idx + 1, :]
                bsrc = bass.AP(tensor=seg.tensor, offset=seg.offset,
                               ap=[[0, 64]] + list(seg.ap))
                dq.dma_start(
                    out=bc[64 * h2:64 * h2 + 64, :].rearrange(
                        "p (a b) -> p a b", a=1),
                    in_=bsrc)

            def drain_mul(pair, j, bc):
                nc.vector.tensor_mul(
                    at_t[pair][:, 512 * j:512 * (j + 1)],
                    at_t[pair][:, 512 * j:512 * (j + 1)],
                    bc[:])

            def drain_block(pair, j):
                """Copy AV out to at_t, recip rowsums, broadcast, normalize."""
                if "nodrain" in skip or "h2o" in skip:
                    return
                bc = bcp.tile([128, 512], F32, tag="bc", name="bc")
                if "dr2" in skip:
                    # store raw rowsum rows straight from PSUM, broadcast
                    # both halves, one recip over [128,512], then normalize
                    for h2 in range(2):
                        cp = (nc.scalar.copy if "atact" in skip
                              else nc.vector.tensor_copy)
                        cp(at_t[pair][64 * h2:64 * h2 + 64,
                                      512 * j:512 * (j + 1)],
                           ot_ps[h2][0:64, :])
                        idx = 4 * j + 2 * pair + h2
                        nc.sync.dma_start(out=rc_dram[idx:idx + 1, :],
                                          in_=ot_ps[h2][64:65, :])
                        seg = rc_dram[idx:idx + 1, :]
                        bsrc = bass.AP(tensor=seg.tensor, offset=seg.offset,
                                       ap=[[0, 64]] + list(seg.ap))
                        nc.sync.dma_start(
                            out=bc[64 * h2:64 * h2 + 64, :].rearrange(
                                "p (a b) -> p a b", a=1),
                            in_=bsrc)
                    nc.vector.reciprocal(out=bc[:], in_=bc[:])
                    nc.vector.tensor_mul(
                        at_t[pair][:, 512 * j:512 * (j + 1)],
                        at_t[pair][:, 512 * j:512 * (j + 1)],
                        bc[:])
                    return
                d0 = 1 if sum0 else 0
                sr = 0 if sum0 else 64   # rowsum partition row in ot
                for h2 in range(2):
                    cp = (nc.scalar.copy if "atact" in skip
                          else nc.vector.tensor_copy)
                    cp(at_t[pair][64 * h2:64 * h2 + 64,
                                  512 * j:512 * (j + 1)],
                       ot_ps[h2][d0:d0 + 64, :])
                    rc = rcp.tile([65, 512], F32, tag="rc", name="rc")
                    if "pbc" in skip and sum0:
                        # partition_broadcast broadcasts partition 0, so it
                        # needs the rowsum (and its reciprocal) on row 0
                        nc.vector.reciprocal(
                            out=rc[0:1, :], in_=ot_ps[h2][0:1, :])
                        nc.gpsimd.partition_broadcast(
                            bc[64 * h2:64 * h2 + 64, :], rc[0:1, :])
                    elif "sbbc" in skip:
                        # broadcast straight from the SBUF rc row with a
                        # partition-stride-0 DMA source (no DRAM hop)
                        nc.vector.reciprocal(
                            out=rc[sr:sr + 1, :],
                            in_=ot_ps[h2][sr:sr + 1, :])
                        src = rc[sr:sr + 1, :]
                        bsrc = bass.AP(tensor=src.tensor, offset=src.offset,
                                       ap=[[0, 64]] + list(src.ap))
                        nc.sync.dma_start(
                            out=bc[64 * h2:64 * h2 + 64, :].rearrange(
                                "p (a b) -> p a b", a=1),
                            in_=bsrc)
                    else:
                        for _ in range(2 if "2xrecip" in skip else 1):
                            nc.vector.reciprocal(
                                out=rc[sr:sr + 1, :],
                                in_=ot_ps[h2][sr:sr + 1, :])
                        idx = 4 * j + 2 * pair + h2
                        dq = nc.gpsimd if "rcpool" in skip else nc.sync
                        dq.dma_start(out=rc_dram[idx:idx + 1, :],
                                     in_=rc[sr:sr + 1, :])
                        seg = rc_dram[idx:idx + 1, :]
                        bsrc = bass.AP(tensor=seg.tensor, offset=seg.offset,
                                       ap=[[0, 64]] + list(seg.ap))
                        dq.dma_start(
                            out=bc[64 * h2:64 * h2 + 64, :].rearrange(
                                "p (a b) -> p a b", a=1),
                            in_=bsrc)
                mul = (nc.gpsimd.tensor_mul if "mulpool" in skip
                       else nc.vector.tensor_mul)
                mul(at_t[pair][:, 512 * j:512 * (j + 1)],
                    at_t[pair][:, 512 * j:512 * (j + 1)],
                    bc[:])

            def outp_unit(j, t, oc):
                        yps = ppp.tile([128, 512], F32, tag="pp", name="pp")
                        for i in range(2):
                            nc.tensor.matmul(
                                yps[:],
                                at_t[i][:, 128 * t:128 * (t + 1)],
                                wo_sb[i][:, 512 * oc:512 * (oc + 1)],
                                start=(i == 0), stop=(i == 1))
                        ysb = ysbp.tile([128, 512], BF16, tag="ysb",
                                        name="ysb")
                        if j == NJ - 1 and oc == 1:
                            # tail: drain on ACT in parallel with DVE
                            nc.scalar.copy(out=ysb[:], in_=yps[:])
                        elif "ysbact" in skip:
                            nc.scalar.copy(out=ysb[:], in_=yps[:])
                        elif "ysbpool" in skip:
                            nc.gpsimd.tensor_copy(ysb[:], yps[:])
                        else:
                            for _ in range(2 if "2xcopy" in skip else 1):
                                nc.vector.tensor_copy(ysb[:], yps[:])
                        (nc.sync if oc == 0 else nc.gpsimd).dma_start(
                            out=y[128 * t:128 * (t + 1),
                                  512 * oc:512 * (oc + 1)],
                            in_=ysb[:])

            def outp(j):
                if "nooutp" in skip:
                    return
                for t in range(4 * j, 4 * j + 4):
                    for oc in range(2):
                        outp_unit(j, t, oc)

            # chunk 0's projections up front; later chunks' projections are
            # slotted between attention groups as PE filler work.
            for m in range(4):
                qk_chunk(m, 0)
            for t in range(4):
                v_tile(t)
            for j in range(NJ):
                if j + 1 < NJ:
                    items = ([(lambda late, m=m: qk_chunk(m, j + 1, late))
                              for m in range(4)]
                             + [(lambda late, t=t: v_tile(t, late))
                                for t in range(4 * j + 4, 4 * j + 8)])
                else:
                    items = []
                if "outpfill" in skip:
                    # out-projection of the previous chunk rides along as
                    # PE filler work inside the attention blocks, keeping
                    # ACT fed with exps while PE does projections.
                    oitems = ([(lambda late, t=t, oc=oc, jj=j - 1:
                                outp_unit(jj, t, oc))
                               for t in range(4 * (j - 1), 4 * (j - 1) + 4)
                               for oc in range(2)] if j > 0 else [])
                    both = []
                    na, nb = len(items), len(oitems)
                    for i in range(max(na, nb)):
                        if i < na:
                            both.append(items[i])
                        if i < nb:
                            both.append(oitems[i])
                    half = (len(both) + 1) // 2
                    dd = att_block(0, j, fillers=both[:half])
                    drain_block(0, j)
                    for f in dd:
                        f()
                    dd = att_block(1, j, fillers=both[half:])
                    drain_block(1, j)
                    for f in dd:
                        f()
                else:
                    half = (len(items) + 1) // 2
                    dd = att_block(0, j, fillers=items[:half])
                    drain_block(0, j)
                    for f in dd:
                        f()
                    if j > 0:
                        outp(j - 1)
                    dd = att_block(1, j, fillers=items[half:])
                    drain_block(1, j)
                    for f in dd:
                        f()
            outp(NJ - 1)
    nc.compile()
    return nc


def _get_runner(reps=1, skip=None):
    """Compile once; return a callable(in_maps) -> list of per-core out dicts."""
    if skip is None:
        skip = DEFAULT_SKIP
    key = ("runner", reps, tuple(skip))
    if key in _CACHE:
        return _CACHE[key]
    import jax
    import jax.numpy as jnp
    from jax.sharding import Mesh, PartitionSpec
    from jax.experimental.shard_map import shard_map
    from concourse import bass2jax

    nc = build_nc(reps, skip)
    bass2jax.install_neuronx_cc_hook()

    partition_name = (nc.partition_id_tensor.name
                      if nc.partition_id_tensor else None)
    in_names, out_names, out_avals, zero_outs = [], [], [], []
    for alloc in nc.m.functions[0].allocations:
        if not isinstance(alloc, mybir.MemoryLocationSet):
            continue
        name = alloc.memorylocations[0].name
        if alloc.kind == "ExternalInput":
            if name != partition_name:
                in_names.append(name)
        elif alloc.kind == "ExternalOutput":
            out_names.append(name)
            shape = tuple(alloc.tensor_shape)
            dtype = mybir.dt.np(alloc.dtype)
            out_avals.append(jax.core.ShapedArray(shape, dtype))
            zero_outs.append(np.zeros(shape, dtype))
    n_params = len(in_names)
    n_outs = len(out_avals)
    all_in_names = list(in_names) + list(out_names)
    if partition_name is not None:
        all_in_names.append(partition_name)
    donate = tuple(range(n_params, n_params + n_outs))

    def _body(*args):
        operands = list(args)
        if partition_name is not None:
            operands.append(bass2jax.partition_id_tensor())
        outs = bass2jax._bass_exec_p.bind(
            *operands,
            out_avals=tuple(out_avals),
            in_names=tuple(all_in_names),
            out_names=tuple(out_names),
            lowering_input_output_aliases=(),
            sim_require_finite=True,
            sim_require_nnan=True,
            nc=nc,
        )
        return tuple(outs)

    n_cores = 8
    devices = jax.devices()[:n_cores]
    mesh = Mesh(np.asarray(devices), ("core",))
    in_specs = (PartitionSpec("core"),) * (n_params + n_outs)
    out_specs = (PartitionSpec("core"),) * n_outs
    sharded = jax.jit(
        shard_map(_body, mesh=mesh, in_specs=in_specs, out_specs=out_specs,
                  check_rep=False),
        donate_argnums=donate, keep_unused=True)

    def run(in_maps):
        per_core = [[np.asarray(m[name]) for name in in_names] for m in in_maps]
        concat_in = [np.concatenate([per_core[c][i] for c in range(n_cores)],
                                    axis=0) for i in range(n_params)]
        concat_zeros = [np.zeros((n_cores * z.shape[0], *z.shape[1:]), z.dtype)
                        for z in zero_outs]
        out_arrs = sharded(*concat_in, *concat_zeros)
        return [
            {name: np.asarray(out_arrs[i]).reshape(n_cores,
                                                   *out_avals[i].shape)[c]
             for i, name in enumerate(out_names)}
            for c in range(n_cores)
        ]

    _CACHE[key] = run
    return run


def _get_bench(reps=1, skip=None):
    """Zero-transfer bench callable: inputs pre-placed on device, outputs
    left on device (block_until_ready only). No donation."""
    if skip is None:
        skip = DEFAULT_SKIP
    key = ("bench", reps, tuple(skip))
    if key in _CACHE:
        return _CACHE[key]
    import jax
    from jax.sharding import Mesh, PartitionSpec, NamedSharding
    from jax.experimental.shard_map import shard_map
    from concourse import bass2jax

    nc = build_nc(reps, skip)
    bass2jax.install_neuronx_cc_hook()
    partition_name = (nc.partition_id_tensor.name
                      if nc.partition_id_tensor else None)
    in_names, out_names, out_avals, zero_outs = [], [], [], []
    for alloc in nc.m.functions[0].allocations:
        if not isinstance(alloc, mybir.MemoryLocationSet):
            continue
        name = alloc.memorylocations[0].name
        if alloc.kind == "ExternalInput":
            if name != partition_name:
                in_names.append(name)
        elif alloc.kind == "ExternalOutput":
            out_names.append(name)
            shape = tuple(alloc.tensor_shape)
            dtype = mybir.dt.np(alloc.dtype)
            out_avals.append(jax.core.ShapedArray(shape, dtype))
            zero_outs.append(np.zeros(shape, dtype))
    n_params = len(in_names)
    all_in_names = list(in_names) + list(out_names)
    if partition_name is not None:
        all_in_names.append(partition_name)

    def _body(*args):
        operands = list(args)
        if partition_name is not None:
            operands.append(bass2jax.partition_id_tensor())
        outs = bass2jax._bass_exec_p.bind(
            *operands,
            out_avals=tuple(out_avals),
            in_names=tuple(all_in_names),
            out_names=tuple(out_names),
            lowering_input_output_aliases=(),
            sim_require_finite=True,
            sim_require_nnan=True,
            nc=nc,
        )
        return tuple(outs)

    n_cores = 8
    devices = jax.devices()[:n_cores]
    mesh = Mesh(np.asarray(devices), ("core",))
    nouts = len(out_names)
    in_specs = (PartitionSpec("core"),) * (n_params + nouts)
    out_specs = (PartitionSpec("core"),) * nouts
    sharded = jax.jit(
        shard_map(_body, mesh=mesh, in_specs=in_specs, out_specs=out_specs,
                  check_rep=False),
        keep_unused=True)
    shard = NamedSharding(mesh, PartitionSpec("core"))

    def make_args(in_maps):
        per_core = [[np.asarray(m[name]) for name in in_names]
                    for m in in_maps]
        concat_in = [np.concatenate([per_core[c][i] for c in range(n_cores)],
                                    axis=0) for i in range(n_params)]
        concat_zeros = [np.zeros((n_cores * z.shape[0], *z.shape[1:]),
                                 z.dtype) for z in zero_outs]
        return [jax.device_put(a, shard) for a in concat_in + concat_zeros]

    def call(dev_args):
        outs = sharded(*dev_args)
        for o in outs:
            o.block_until_ready()
        return outs

    call.sharded = sharded
    result = (make_args, call)
    _CACHE[key] = result
    return result


def _prep_in_maps(x, w_qkv, w_out, skip=None):
    if skip is None:
        skip = DEFAULT_SKIP
    import ml_dtypes
    bf = ml_dtypes.bfloat16
    x = np.asarray(x, dtype=np.float32)
    w_qkv = np.asarray(w_qkv, dtype=np.float32)
    w_out = np.asarray(w_out, dtype=np.float32)
    msk = np.zeros((128, 256), dtype=np.float32)
    msk[:, 0:128] = np.triu(np.full((128, 128), NEG, dtype=np.float32), k=1)
    msk[:, 128:256] = np.eye(128, dtype=np.float32)
    msk = msk.astype(bf)
    in_maps = []
    xts = [np.ascontiguousarray(x[b].T).astype(bf) for b in range(B)]

    for core in range(8):
        b, g = divmod(core, 4)
        cl, ch = 256 * g, 256 * g + 256
        wqkv = np.ascontiguousarray(np.concatenate(
            [w_qkv[:, cl:ch], w_qkv[:, C + cl:C + ch],
             w_qkv[:, 2 * C + cl:2 * C + ch]], axis=1)).astype(bf)
        wo = np.ascontiguousarray(w_out[cl:ch, :]).astype(bf)
        in_maps.append({"xt": xts[b], "wqkv": wqkv, "wo": wo, "msk": msk})
    return in_maps


def kernel(x, w_qkv, w_out):
    run = _get_runner()
    in_maps = _prep_in_maps(x, w_qkv, w_out)
    results = run(in_maps)
    y = np.zeros((B, T, C), dtype=np.float32)
    for core in range(8):
        b = core // 4
        y[b] += results[core]["y"].astype(np.float32)
    return y


if __name__ == "__main__":
    rng = np.random.default_rng(0)
    x = rng.standard_normal((B, T, C)).astype(np.float32)
    w_qkv = (rng.standard_normal((C, 3 * C)) / np.sqrt(C)).astype(np.float32)
    w_out = (rng.standard_normal((C, C)) / np.sqrt(C)).astype(np.float32)
    y = kernel(x=x, w_qkv=w_qkv, w_out=w_out)
    print("kernel ran, y:", y.shape, y.dtype, float(np.abs(y).max()))



# revision 67
# speedup vs baseline: 1.0584x; 1.0584x over previous
"""Trainium2 Bass kernel for multi-head causal attention (v3, bf16+fp8).

Problem (hardcoded): x [2, 2048, 1024] fp32, w_qkv [1024, 3072], w_out
[1024, 1024].
  qkv = x @ w_qkv; per-head causal softmax attention (16 heads, d=64);
  out = attn_out @ w_out.

Sharding: 8 cores = (2 batches) x (4 head-groups of 4 heads).
Each core computes, for its batch b and heads 4g..4g+3 (2 pairs of 2 heads):
  - Q^T, K^T [256, 2048] and V [2048, 256] from x[b]^T (host-pretransposed,
    bf16) via PE, pipelined as filler work between attention groups
  - causal attention on-chip in S^T layout (S in bf16 — fp8 QK noise breaks
    tolerance); diagonal tiles are column-sliced so fully-masked columns are
    never computed, the remaining 128-col triangle zeroed post-exp by a
    gpsimd affine_select; rowsum via a ones-column in the AV stationary
  - AV for q-chunks j>=1 runs in fp8e4 DoubleRow (2 k-tiles per matmul at
    0.5 cyc/col; exp writes p2t as fp8 with a -4 logit shift so the
    numerator can't overflow e4m3's 448 max — the shift cancels in the
    normalize).  Chunk 0 keeps the bf16 path: with <512 keys the fp8 P/V
    quantization noise doesn't average out.
  - per-head group ordering (h2-outer): each head's AV bank drains (at-copy
    on ACT, reciprocal on DVE, DRAM-hop broadcast DMA) while the other
    head's attention still runs; normalization mul on DVE once both
    broadcast halves land
  - out-projection + drains pipelined against the next block; attention
    tiles parity-alternate between reps so a benched rep's projections
    overlap the previous rep's tail
Host gathers: y[b] = sum_g y_part[4b+g] (partials stored bf16).
"""
import numpy as np

import concourse.bass as bass
from concourse import bacc
import concourse.mybir as mybir
import concourse.tile as tile

F32 = mybir.dt.float32
F32R = mybir.dt.float32r
BF16 = mybir.dt.bfloat16
FP8 = mybir.dt.float8e4
AF = mybir.ActivationFunctionType
DR = mybir.MatmulPerfMode.DoubleRow

B, T, C = 2, 2048, 1024
H_TOT, D = 16, 64
HL = 4             # heads per core
DL = HL * D        # 256 local channels
NJ = 4             # q-chunks of 512
NKT = 16           # k-tiles of 128
NCT = 8            # c-tiles of 128 (contraction over C)
SM_SCALE = 1.0 / np.sqrt(D)
NEG = -30000.0

_CACHE = {}

# variant used by kernel() and by test.py's default bench:
#   fp8   - AV via fp8e4 DoubleRow for chunks j>=1 (chunk 0 stays bf16)
#   dexp1 - single-span exp on diagonal groups
#   xrep  - parity-alternated attention tiles (cross-rep overlap)
#   atact - drain PSUM->SBUF copies on ACT (frees the DVE release path)
#   h2o   - per-head group ordering with early per-head drains
#   pend2 - AV emission lags S/exp by 2 groups
#   dr3   - broadcast the raw rowsum; one post-DMA reciprocal on the
#           broadcast result keeps DVE off the PSUM-release path
DEFAULT_SKIP = ("fp8", "dexp1", "xrep", "atact", "h2o", "pend2", "dr3")


def build_nc(reps=1, skip=()):
    nc = bacc.Bacc("TRN2", target_bir_lowering=False)
    xt = nc.dram_tensor("xt", [C, T], BF16, kind="ExternalInput")
    wqkv = nc.dram_tensor("wqkv", [C, 3 * DL], BF16, kind="ExternalInput")
    wo = nc.dram_tensor("wo", [DL, C], BF16, kind="ExternalInput")
    msk = nc.dram_tensor("msk", [128, 256], BF16, kind="ExternalInput")
    y = nc.dram_tensor("y", [T, C], BF16, kind="ExternalOutput")
    if "rcout" in skip:
        rcdbg = nc.dram_tensor("rcdbg", [16, 512], F32, kind="ExternalOutput")

    with tile.TileContext(nc) as tc:
      for _rep in range(reps):
        with tc.tile_pool(name="persist", bufs=1) as persist, \
             tc.tile_pool(name="dram", bufs=1, space="DRAM") as drampool, \
             tc.tile_pool(name="rcp", bufs=6) as rcp, \
             tc.tile_pool(name="bcp", bufs=3) as bcp, \
             tc.tile_pool(name="pexp", bufs=6) as pexp, \
             tc.tile_pool(name="ysbp", bufs=4) as ysbp, \
             tc.tile_pool(name="s2p",
                          bufs=1 if "bigexp" in skip else 2,
                          space="PSUM") as s2p, \
             tc.tile_pool(name="otp", bufs=1, space="PSUM") as otp, \
             tc.tile_pool(name="ppp", bufs=2, space="PSUM") as ppp:
            use8 = "fp8" in skip
            # cross-rep parity: alternate the attention tiles between reps
            # so rep N+1's projections aren't WAR-blocked on rep N's last
            # S/AV reads (benched steady-state overlaps rep tails)
            par = _rep % 2 if "xrep" in skip else 0
            qk_tiles = [persist.tile([128, T], BF16, tag=f"qk{m}_{par}",
                                     name=f"qk{m}_{par}") for m in range(4)]
            qt_t, kt_t = qk_tiles[0:2], qk_tiles[2:4]
            # S stays bf16 (fp8 QK noise breaks tolerance); AV for chunks
            # j>=1 uses fp8 DoubleRow (chunk 0 has too few keys to average
            # out fp8 P/V noise, so it keeps the bf16 path and only needs
            # bf16 V for k-tiles 0..3).
            nvsb = 4 if use8 else NKT
            v_sb = [persist.tile([128, HL, D + 1], BF16, tag=f"v{t}_{par}",
                                 name=f"v{t}_{par}") for t in range(nvsb)]
            if use8:
                # vp8 per k-tile pair [128p, 2 ktile, HL, 128] (col D = ones
                # rowsum, col D+1 pad: stationary 66 wide; per-head width
                # padded to 128 so the DoubleRow stationary's k-subtile
                # stride is 512B, an ISA restriction)
                vp8 = [persist.tile([128, 2, HL, 128], FP8,
                                    tag=f"vp{t}_{par}",
                                    name=f"vp{t}_{par}")
                       for t in range(NKT // 2)]
                ebias = persist.tile([128, 1], F32, tag="eb", name="eb")
                nc.vector.memset(ebias[:], -4.0)
            at_t = [persist.tile([128, T], BF16, tag=f"at{p}_{par}",
                                 name=f"at{p}_{par}") for p in range(2)]
            xt_sb = [persist.tile([128, T], BF16, tag=f"xt{c}", name=f"xt{c}")
                     for c in range(NCT)]
            wqkv_sb = [persist.tile([128, 3 * DL], BF16, tag=f"wq{c}",
                                    name=f"wq{c}") for c in range(NCT)]
            wo_sb = [persist.tile([128, C], BF16, tag=f"wo{i}", name=f"wo{i}")
                     for i in range(2)]
            msk_sb = persist.tile([128, 256], BF16, tag="msk", name="msk")
            otd = 66 if use8 else 65
            ot_ps = [otp.tile([otd, 512], F32, tag=f"ot{h2}", name=f"ot{h2}")
                     for h2 in range(2)]
            if "rcout" in skip:
                rc_dram = rcdbg[:, :]
            else:
                rc_dram = drampool.tile([16, 512], F32)

            if "probe" in skip:
                pr = persist.tile([128, 2, 512], FP8, tag="pr8", name="pr8")
                pr2 = persist.tile([128, 2, 65], FP8, tag="pr8b", name="pr8b")
                prq = persist.tile([128, 2, 512], FP8, tag="pr8c", name="pr8c")
                prps = ppp.tile([128, 512], F32, tag="pp", name="pp")
                nc.tensor.matmul(prps[:], wqkv_sb[0][:, 0:128],
                                 xt_sb[0][:, 0:512], start=True, stop=True)
                # (a) DVE f32->fp8 cast from PSUM
                nc.vector.tensor_copy(pr[:, 0, :], prps[:])
                # (b) ACT exp with fp8 out
                nc.scalar.activation(pr[:, 1, :], prps[:], AF.Exp, scale=0.01)
                # (c) affine_select on fp8
                nc.gpsimd.affine_select(
                    out=pr[:, 0, 0:128], in_=pr[:, 0, 0:128],
                    compare_op=mybir.AluOpType.is_ge, fill=0.0, base=0,
                    pattern=[[1, 128]], channel_multiplier=-1)
                # (e) fp8 memset
                nc.vector.memset(pr2[:, :, 64:65], 1.0)
                nc.vector.tensor_copy(pr2[:, 0, 0:64], prps[:, 0:64])
                nc.vector.tensor_copy(prq[:, :, 0:256],
                                      prps[:].rearrange("p (k q) -> p k q",
                                                        k=2))
                ps8 = ppp.tile([128, 512], F32, tag="pp", name="pp")
                if "pa" in skip:   # AV-style DoubleRow, M=65 (odd)
                    nc.tensor.matmul(ps8[0:65, :], pr2[:, :, :], pr[:, :, :],
                                     start=True, stop=True, perf_mode=DR)
                if "pb" in skip:   # AV-style DoubleRow, M=64
                    nc.tensor.matmul(ps8[0:64, :], pr2[:, :, 0:64],
                                     pr[:, :, :], start=True, stop=True,
                                     perf_mode=DR)
                if "pc" in skip:   # AV-style DoubleRow, M=66 via pr tile
                    nc.tensor.matmul(ps8[0:66, :], pr[:, :, 0:66],
                                     pr[:, :, :], start=True, stop=True,
                                     perf_mode=DR)
                if "pd" in skip:   # S-style 32-row DoubleRow at base 32
                    nc.tensor.matmul(ps8[:, 0:512], prq[32:64, :, 0:128],
                                     prq[32:64, :, :],
                                     start=True, stop=True, perf_mode=DR)
                if "pe2" in skip:  # mixed accumulation group DR + plain fp8
                    nc.tensor.matmul(ps8[0:66, :], pr[:, :, 0:66],
                                     pr[:, :, :], start=True, stop=False,
                                     perf_mode=DR)
                    nc.tensor.matmul(ps8[0:66, 128:512], pr[:, 0, 0:66],
                                     pr[:, 0, 128:512], start=False,
                                     stop=True)
                nc.vector.tensor_copy(at_t[0][:, 0:512], ps8[:])

            # ---- loads ----
            # xt column-split: chunk-0 columns first so projections for j=0
            # can start before the bulk of x arrives.
            for c in range(NCT):
                eng = nc.sync if c % 2 == 0 else nc.scalar
                eng.dma_start(out=xt_sb[c][:, 0:512],
                              in_=xt[128 * c:128 * (c + 1), 0:512])
                (nc.scalar if c % 2 == 0 else nc.sync).dma_start(
                    out=wqkv_sb[c][:], in_=wqkv[128 * c:128 * (c + 1), :])
            for c in range(NCT):
                (nc.sync if c % 2 == 0 else nc.scalar).dma_start(
                    out=xt_sb[c][:, 512:T],
                    in_=xt[128 * c:128 * (c + 1), 512:T])
            for i in range(2):
                nc.sync.dma_start(out=wo_sb[i][:],
                                  in_=wo[128 * i:128 * (i + 1), :])
            nc.scalar.dma_start(out=msk_sb[:], in_=msk[:, :])
            sum0 = "sum0" in skip
            vco = 1 if sum0 else 0     # v data column offset in stationaries
            if use8:
                for tp in range(NKT // 2):
                    if sum0:
                        nc.vector.memset(vp8[tp][:, :, :, 0:1], 1.0)
                        nc.vector.memset(vp8[tp][:, :, :, 65:66], 1.0)
                    else:
                        nc.vector.memset(vp8[tp][:, :, :, D:D + 2], 1.0)
            for t in range(nvsb):
                if sum0:
                    nc.vector.memset(v_sb[t][:, :, 0:1], 1.0)
                else:
                    nc.vector.memset(v_sb[t][:, :, D:D + 1], 1.0)

            def qk_chunk(m, j, defer=False):
                ps = ppp.tile([128, 512], F32, tag="pp", name="pp")
                for c in range(NCT):
                    nc.tensor.matmul(
                        ps[:],
                        wqkv_sb[c][:, 128 * m:128 * (m + 1)],
                        xt_sb[c][:, 512 * j:512 * (j + 1)],
                        start=(c == 0), stop=(c == NCT - 1))

                def copy():
                    nc.vector.tensor_copy(
                        qk_tiles[m][:, 512 * j:512 * (j + 1)], ps[:])
                if defer:
                    return copy
                copy()

            def v_tile(t, defer=False):
                ps = ppp.tile([128, 512], F32, tag="pp", name="pp")
                for c in range(NCT):
                    nc.tensor.matmul(
                        ps[:, 0:DL],
                        xt_sb[c][:, 128 * t:128 * (t + 1)],
                        wqkv_sb[c][:, 2 * DL:3 * DL],
                        start=(c == 0), stop=(c == NCT - 1))

                def copy():
                    if use8:
                        nc.vector.tensor_copy(
                            vp8[t // 2][:, t % 2, :, vco:vco + D],
                            ps[:, 0:DL].rearrange("p (h d) -> p h d", h=HL))
                        if t < nvsb:
                            nc.vector.tensor_copy(
                                v_sb[t][:, :, vco:vco + D],
                                ps[:, 0:DL].rearrange("p (h d) -> p h d",
                                                      h=HL))
                    else:
                        nc.vector.tensor_copy(
                            v_sb[t][:, :, vco:vco + D],
                            ps[:, 0:DL].rearrange("p (h d) -> p h d", h=HL))
                if defer:
                    return copy
                copy()

            def s_group(pair, j, ktg, h2):
                """Emit S^T matmuls (+pre-exp causal mask) for one s2 group
                (2 k-tiles); returns the s2 tile and exp metadata."""
                s2 = s2p.tile([128, 1024], F32, tag="s", name="s")
                diag = (ktg >= 2 * j)
                segs = []
                for kk in range(2):
                    ktt = 2 * ktg + kk
                    col0 = 512 * kk
                    q0 = 0 if not diag else 128 * (ktt - 4 * j)
                    qt, kt = qt_t[pair], kt_t[pair]
                    base = 64 * h2
                    nc.tensor.matmul(
                        s2[:, col0 + q0:col0 + 512],
                        kt[base:base + 64, 128 * ktt:128 * (ktt + 1)],
                        qt[base:base + 64, 512 * j + q0:512 * (j + 1)],
                        start=True, stop=True)
                    segs.append((col0 + q0, 512 - q0))
                return s2, diag, segs

            def exp_group(s2, diag, segs, p8):
                p2t = pexp.tile([128, 1024], FP8 if p8 else BF16,
                                tag="p8" if p8 else "p",
                                name="p8" if p8 else "p")
                if "noexp" in skip:
                    return p2t
                # fp8e4 has no inf and max 448: shift logits down so the
                # softmax numerator never overflows (cancels in normalize)
                eb = ebias[:, :] if p8 else 0.0
                nrep = 2 if "2xexp" in skip else 1
                if not diag:
                    for _ in range(nrep):
                        nc.scalar.activation(p2t[:], s2[:], AF.Exp,
                                             scale=float(SM_SCALE), bias=eb)
                elif "dexp1" in skip:
                    # single span including the dead gap between segments
                    col_lo = segs[0][0]
                    nc.scalar.activation(p2t[:, col_lo:1024],
                                         s2[:, col_lo:1024], AF.Exp,
                                         scale=float(SM_SCALE), bias=eb)
                else:
                    for col0, w in segs:
                        for _ in range(nrep):
                            nc.scalar.activation(p2t[:, col0:col0 + w],
                                                 s2[:, col0:col0 + w], AF.Exp,
                                                 scale=float(SM_SCALE),
                                                 bias=eb)
                return p2t

            def av_group(pair, j, ktg, h2, p2t, diag, segs):
                """ot bank group: start=True only on the block's first matmul
                (clears the bank), stop=True only on its last (r=3 part A)."""
                h = 2 * pair + h2
                ot = ot_ps[h2]
                p8 = use8 and j >= 1
                if p8 and not diag:
                    # one DoubleRow matmul covers both k-tiles of the group
                    nc.tensor.matmul(
                        ot[:, 0:512], vp8[ktg][:, :, h, 0:D + 2],
                        p2t[:].rearrange("p (k q) -> p k q", k=2),
                        start=(ktg == 0), stop=False, perf_mode=DR)
                    return
                for kk in range(2):
                    ktt = 2 * ktg + kk
                    col0 = 512 * kk
                    if p8:
                        vv = vp8[ktg][:, kk, h, 0:D + 2]
                        ot = ot_ps[h2]
                    else:
                        vv = v_sb[ktt][:, h, 0:D + 1]
                        ot = ot_ps[h2][0:65]
                    if not diag:
                        nc.tensor.matmul(ot[:, 0:512], vv,
                                         p2t[:, col0:col0 + 512],
                                         start=(ktt == 0), stop=False)
                        if "2xav" in skip:
                            nc.tensor.matmul(ot[:, 0:512], vv,
                                             p2t[:, col0:col0 + 512],
                                             start=False, stop=False)
                    else:
                        r = ktt - 4 * j
                        q0 = 128 * r
                        if "nomask" not in skip:
                            nc.gpsimd.affine_select(
                                out=p2t[:, col0 + q0:col0 + q0 + 128],
                                in_=p2t[:, col0 + q0:col0 + q0 + 128],
                                compare_op=mybir.AluOpType.is_ge,
                                fill=0.0, base=0,
                                pattern=[[1, 128]],
                                channel_multiplier=-1)
                        first = (j == 0 and r == 0)
                        nc.tensor.matmul(ot[:, q0:512], vv,
                                         p2t[:, col0 + q0:col0 + 512],
                                         start=first, stop=(r == 3))

            def att_block(pair, j, fillers=()):
                """Software-pipelined S -> exp -> AV over all groups.
                `fillers` are independent emission closures (projections for
                the next chunk) slotted between groups to keep PE fed while
                the exp chain runs."""
                fillers = list(fillers)
                deferred = []
                fdef = "fdefer" in skip
                lag = 3 if "pend3" in skip else (2 if "pend2" in skip else 1)
                if "bigexp" in skip:
                    # super-groups of 4 k-tiles over one 4-bank s4 tile;
                    # a single 2048-wide exp per super-group (h2-outer)
                    bc = bcp.tile([128, 512], F32, tag="bc", name="bc")
                    nsg = j + 1
                    p8 = use8 and j >= 1

                    def av_sg(h2, sg, p4, diag_sg):
                        h = 2 * pair + h2
                        for kk in range(4):
                            ktt = 4 * sg + kk
                            col0 = 512 * kk
                            if diag_sg:
                                r = ktt - 4 * j
                                q0 = 128 * r
                                nc.gpsimd.affine_select(
                                    out=p4[:, col0 + q0:col0 + q0 + 128],
                                    in_=p4[:, col0 + q0:col0 + q0 + 128],
                                    compare_op=mybir.AluOpType.is_ge,
                                    fill=0.0, base=0, pattern=[[1, 128]],
                                    channel_multiplier=-1)
                                if p8:
                                    vv = vp8[2 * sg + kk // 2][
                                        :, kk % 2, h, 0:D + 2]
                                    oto = ot_ps[h2]
                                else:
                                    vv = v_sb[ktt][:, h, 0:D + 1]
                                    oto = ot_ps[h2][0:65]
                                first = (j == 0 and r == 0)
                                nc.tensor.matmul(
                                    oto[:, q0:512], vv,
                                    p4[:, col0 + q0:col0 + 512],
                                    start=first, stop=(r == 3))
                            elif p8:
                                if kk % 2 == 1:
                                    continue
                                ktg = 2 * sg + kk // 2
                                nc.tensor.matmul(
                                    ot_ps[h2][:, 0:512],
                                    vp8[ktg][:, :, h, 0:D + 2],
                                    p4[:, col0:col0 + 1024].rearrange(
                                        "p (k q) -> p k q", k=2),
                                    start=(ktg == 0), stop=False,
                                    perf_mode=DR)
                            else:
                                nc.tensor.matmul(
                                    ot_ps[h2][0:65, 0:512],
                                    v_sb[ktt][:, h, 0:D + 1],
                                    p4[:, col0:col0 + 512],
                                    start=(ktt == 0), stop=False)

                    for h2 in range(2):
                        pend = []
                        for sg in range(nsg):
                            diag_sg = (sg == j)
                            s4 = s2p.tile([128, 2048], F32, tag="s",
                                          name="s")
                            for kk in range(4):
                                ktt = 4 * sg + kk
                                col0 = 512 * kk
                                r = ktt - 4 * j
                                q0 = 128 * r if (diag_sg and r > 0) else 0
                                qt, kt = qt_t[pair], kt_t[pair]
                                base = 64 * h2
                                nc.tensor.matmul(
                                    s4[:, col0 + q0:col0 + 512],
                                    kt[base:base + 64,
                                       128 * ktt:128 * (ktt + 1)],
                                    qt[base:base + 64,
                                       512 * j + q0:512 * (j + 1)],
                                    start=True, stop=True)
                            p4 = pexp.tile([128, 2048],
                                           FP8 if p8 else BF16,
                                           tag="p8" if p8 else "p",
                                           name="p8" if p8 else "p")
                            eb = ebias[:, :] if p8 else 0.0
                            nc.scalar.activation(p4[:], s4[:], AF.Exp,
                                                 scale=float(SM_SCALE),
                                                 bias=eb)
                            pend.append((sg, p4, diag_sg))
                            if fillers:
                                d = fillers.pop(0)(False)
                                if d is not None:
                                    deferred.append(d)
                            if len(pend) > 1:
                                g = pend.pop(0)
                                av_sg(h2, g[0], g[1], g[2])
                        for g in pend:
                            av_sg(h2, g[0], g[1], g[2])
                        drain_h2(pair, j, h2, bc)
                    drain_mul(pair, j, bc)
                    for f in fillers:
                        d = f(False)
                        if d is not None:
                            deferred.append(d)
                    return deferred
                if "h2o" in skip:
                    # h2-outer: finish one head's groups (and its drain)
                    # before the other's, so each ot bank is released and
                    # normalized mid-block instead of both at block end
                    bc = bcp.tile([128, 512], F32, tag="bc", name="bc")
                    ng = 2 * (j + 1)
                    for h2 in range(2):
                        pend = []
                        for ktg in range(ng):
                            s2, diag, segs = s_group(pair, j, ktg, h2)
                            p2t = exp_group(s2, diag, segs, use8 and j >= 1)
                            pend.append((ktg, h2, p2t, diag, segs))
                            if fillers:
                                late = fdef and (h2 == 1 and ktg >= ng - 3)
                                d = fillers.pop(0)(late)
                                if d is not None:
                                    deferred.append(d)
                            if len(pend) > lag:
                                g = pend.pop(0)
                                av_group(pair, j, g[0], g[1], g[2], g[3],
                                         g[4])
                        for g in pend:
                            av_group(pair, j, g[0], g[1], g[2], g[3], g[4])
                        drain_h2(pair, j, h2, bc)
                    drain_mul(pair, j, bc)
                    for f in fillers:
                        d = f(fdef)
                        if d is not None:
                            deferred.append(d)
                    return deferred
                glist = [(ktg, h2) for ktg in range(2 * (j + 1))
                         for h2 in range(2)]
                pend = []   # (ktg, h2, p2t, diag, segs) awaiting AV emission
                for i, (ktg, h2) in enumerate(glist):
                    s2, diag, segs = s_group(pair, j, ktg, h2)
                    p2t = exp_group(s2, diag, segs, use8 and j >= 1)
                    pend.append((ktg, h2, p2t, diag, segs))
                    if fillers:
                        late = fdef and i >= len(glist) - 3
                        d = fillers.pop(0)(late)
                        if d is not None:
                            deferred.append(d)
                    if i >= lag:
                        g = pend.pop(0)
                        av_group(pair, j, g[0], g[1], g[2], g[3], g[4])
                for g in pend:
                    av_group(pair, j, g[0], g[1], g[2], g[3], g[4])
                for f in fillers:
                    d = f(fdef)
                    if d is not None:
                        deferred.append(d)
                return deferred

            def drain_h2(pair, j, h2, bc):
                """Per-head drain: at-copy + recip + broadcast into bc half."""
                cp = (nc.scalar.copy if "atact" in skip
                      else nc.vector.tensor_copy)
                cp(at_t[pair][64 * h2:64 * h2 + 64,
                              512 * j:512 * (j + 1)],
                   ot_ps[h2][0:64, :])
                rc = rcp.tile([65, 512], F32, tag="rc", name="rc")
                if "dr3" in skip:
                    # broadcast the RAW rowsum; reciprocal happens once on
                    # the broadcast result (drain_mul), so the DVE is fully
                    # off the PSUM-release path
                    nc.scalar.copy(out=rc[64:65, :],
                                   in_=ot_ps[h2][64:65, :])
                else:
                    nc.vector.reciprocal(out=rc[64:65, :],
                                         in_=ot_ps[h2][64:65, :])
                idx = 4 * j + 2 * pair + h2
                dq = nc.gpsimd if "rcpool" in skip else nc.sync
                dq.dma_start(out=rc_dram[idx:idx + 1, :], in_=rc[64:65, :])
                seg = rc_dram[idx:idx + 1, :]
                bsrc = bass.AP(tensor=seg.tensor, offset=seg.offset,
                               ap=[[0, 64]] + list(seg.ap))
                dq.dma_start(
                    out=bc[64 * h2:64 * h2 + 64, :].rearrange(
                        "p (a b) -> p a b", a=1),
                    in_=bsrc)

            def drain_mul(pair, j, bc):
                if "dr3" in skip:
                    nc.vector.reciprocal(out=bc[:], in_=bc[:])
                nc.vector.tensor_mul(
                    at_t[pair][:, 512 * j:512 * (j + 1)],
                    at_t[pair][:, 512 * j:512 * (j + 1)],
                    bc[:])

            def drain_block(pair, j):
                """Copy AV out to at_t, recip rowsums, broadcast, normalize."""
                if "nodrain" in skip or "h2o" in skip:
                    return
                bc = bcp.tile([128, 512], F32, tag="bc", name="bc")
                if "dr2" in skip:
                    # store raw rowsum rows straight from PSUM, broadcast
                    # both halves, one recip over [128,512], then normalize
                    for h2 in range(2):
                        cp = (nc.scalar.copy if "atact" in skip
                              else nc.vector.tensor_copy)
                        cp(at_t[pair][64 * h2:64 * h2 + 64,
                                      512 * j:512 * (j + 1)],
                           ot_ps[h2][0:64, :])
                        idx = 4 * j + 2 * pair + h2
                        nc.sync.dma_start(out=rc_dram[idx:idx + 1, :],
                                          in_=ot_ps[h2][64:65, :])
                        seg = rc_dram[idx:idx + 1, :]
                        bsrc = bass.AP(tensor=seg.tensor, offset=seg.offset,
                                       ap=[[0, 64]] + list(seg.ap))
                        nc.sync.dma_start(
                            out=bc[64 * h2:64 * h2 + 64, :].rearrange(
                                "p (a b) -> p a b", a=1),
                            in_=bsrc)
                    nc.vector.reciprocal(out=bc[:], in_=bc[:])
                    nc.vector.tensor_mul(
                        at_t[pair][:, 512 * j:512 * (j + 1)],
                        at_t[pair][:, 512 * j:512 * (j + 1)],
                        bc[:])
                    return
                d0 = 1 if sum0 else 0
                sr = 0 if sum0 else 64   # rowsum partition row in ot
                for h2 in range(2):
                    cp = (nc.scalar.copy if "atact" in skip
                          else nc.vector.tensor_copy)
                    cp(at_t[pair][64 * h2:64 * h2 + 64,
                                  512 * j:512 * (j + 1)],
                       ot_ps[h2][d0:d0 + 64, :])
                    rc = rcp.tile([65, 512], F32, tag="rc", name="rc")
                    if "pbc" in skip and sum0:
                        # partition_broadcast broadcasts partition 0, so it
                        # needs the rowsum (and its reciprocal) on row 0
                        nc.vector.reciprocal(
                            out=rc[0:1, :], in_=ot_ps[h2][0:1, :])
                        nc.gpsimd.partition_broadcast(
                            bc[64 * h2:64 * h2 + 64, :], rc[0:1, :])
                    elif "sbbc" in skip:
                        # broadcast straight from the SBUF rc row with a
                        # partition-stride-0 DMA source (no DRAM hop)
                        nc.vector.reciprocal(
                            out=rc[sr:sr + 1, :],
                            in_=ot_ps[h2][sr:sr + 1, :])
                        src = rc[sr:sr + 1, :]
                        bsrc = bass.AP(tensor=src.tensor, offset=src.offset,
                                       ap=[[0, 64]] + list(src.ap))
                        nc.sync.dma_start(
                            out=bc[64 * h2:64 * h2 + 64, :].rearrange(
                                "p (a b) -> p a b", a=1),
                            in_=bsrc)
                    else:
                        for _ in range(2 if "2xrecip" in skip else 1):
                            nc.vector.reciprocal(
                                out=rc[sr:sr + 1, :],
                                in_=ot_ps[h2][sr:sr + 1, :])
                        idx = 4 * j + 2 * pair + h2
                        dq = nc.gpsimd if "rcpool" in skip else nc.sync
                        dq.dma_start(out=rc_dram[idx:idx + 1, :],
                                     in_=rc[sr:sr + 1, :])
                        seg = rc_dram[idx:idx + 1, :]
                        bsrc = bass.AP(tensor=seg.tensor, offset=seg.offset,
                                       ap=[[0, 64]] + list(seg.ap))
                        dq.dma_start(
                            out=bc[64 * h2:64 * h2 + 64, :].rearrange(
                                "p (a b) -> p a b", a=1),
                            in_=bsrc)
                mul = (nc.gpsimd.tensor_mul if "mulpool" in skip
                       else nc.vector.tensor_mul)
                mul(at_t[pair][:, 512 * j:512 * (j + 1)],
                    at_t[pair][:, 512 * j:512 * (j + 1)],
                    bc[:])

            def outp_unit(j, t, oc):
                        yps = ppp.tile([128, 512], F32, tag="pp", name="pp")
                        for i in range(2):
                            nc.tensor.matmul(
                                yps[:],
                                at_t[i][:, 128 * t:128 * (t + 1)],
                                wo_sb[i][:, 512 * oc:512 * (oc + 1)],
                                start=(i == 0), stop=(i == 1))
                        ysb = ysbp.tile([128, 512], BF16, tag="ysb",
                                        name="ysb")
                        if j == NJ - 1 and oc == 1:
                            # tail: drain on ACT in parallel with DVE
                            nc.scalar.copy(out=ysb[:], in_=yps[:])
                        elif "ysbact" in skip:
                            nc.scalar.copy(out=ysb[:], in_=yps[:])
                        elif "ysbpool" in skip:
                            nc.gpsimd.tensor_copy(ysb[:], yps[:])
                        else:
                            for _ in range(2 if "2xcopy" in skip else 1):
                                nc.vector.tensor_copy(ysb[:], yps[:])
                        (nc.sync if oc == 0 else nc.gpsimd).dma_start(
                            out=y[128 * t:128 * (t + 1),
                                  512 * oc:512 * (oc + 1)],
                            in_=ysb[:])

            def outp(j):
                if "nooutp" in skip:
                    return
                for t in range(4 * j, 4 * j + 4):
                    for oc in range(2):
                        outp_unit(j, t, oc)

            # chunk 0's projections up front; later chunks' projections are
            # slotted between attention groups as PE filler work.
            for m in range(4):
                qk_chunk(m, 0)
            for t in range(4):
                v_tile(t)
            for j in range(NJ):
                if j + 1 < NJ:
                    items = ([(lambda late, m=m: qk_chunk(m, j + 1, late))
                              for m in range(4)]
                             + [(lambda late, t=t: v_tile(t, late))
                                for t in range(4 * j + 4, 4 * j + 8)])
                else:
                    items = []
                if "outpfill" in skip:
                    # out-projection of the previous chunk rides along as
                    # PE filler work inside the attention blocks, keeping
                    # ACT fed with exps while PE does projections.
                    oitems = ([(lambda late, t=t, oc=oc, jj=j - 1:
                                outp_unit(jj, t, oc))
                               for t in range(4 * (j - 1), 4 * (j - 1) + 4)
                               for oc in range(2)] if j > 0 else [])
                    both = []
                    na, nb = len(items), len(oitems)
                    for i in range(max(na, nb)):
                        if i < na:
                            both.append(items[i])
                        if i < nb:
                            both.append(oitems[i])
                    half = (len(both) + 1) // 2
                    dd = att_block(0, j, fillers=both[:half])
                    drain_block(0, j)
                    for f in dd:
                        f()
                    dd = att_block(1, j, fillers=both[half:])
                    drain_block(1, j)
                    for f in dd:
                        f()
                else:
                    half = (len(items) + 1) // 2
                    dd = att_block(0, j, fillers=items[:half])
                    drain_block(0, j)
                    for f in dd:
                        f()
                    if j > 0:
                        outp(j - 1)
                    dd = att_block(1, j, fillers=items[half:])
                    drain_block(1, j)
                    for f in dd:
                        f()
            outp(NJ - 1)
    nc.compile()
    return nc


def _get_runner(reps=1, skip=None):
    """Compile once; return a callable(in_maps) -> list of per-core out dicts."""
    if skip is None:
        skip = DEFAULT_SKIP
    key = ("runner", reps, tuple(skip))
    if key in _CACHE:
        return _CACHE[key]
    import jax
    import jax.numpy as jnp
    from jax.sharding import Mesh, PartitionSpec
    from jax.experimental.shard_map import shard_map
    from concourse import bass2jax

    nc = build_nc(reps, skip)
    bass2jax.install_neuronx_cc_hook()

    partition_name = (nc.partition_id_tensor.name
                      if nc.partition_id_tensor else None)
    in_names, out_names, out_avals, zero_outs = [], [], [], []
    for alloc in nc.m.functions[0].allocations:
        if not isinstance(alloc, mybir.MemoryLocationSet):
            continue
        name = alloc.memorylocations[0].name
        if alloc.kind == "ExternalInput":
            if name != partition_name:
                in_names.append(name)
        elif alloc.kind == "ExternalOutput":
            out_names.append(name)
            shape = tuple(alloc.tensor_shape)
            dtype = mybir.dt.np(alloc.dtype)
            out_avals.append(jax.core.ShapedArray(shape, dtype))
            zero_outs.append(np.zeros(shape, dtype))
    n_params = len(in_names)
    n_outs = len(out_avals)
    all_in_names = list(in_names) + list(out_names)
    if partition_name is not None:
        all_in_names.append(partition_name)
    donate = tuple(range(n_params, n_params + n_outs))

    def _body(*args):
        operands = list(args)
        if partition_name is not None:
            operands.append(bass2jax.partition_id_tensor())
        outs = bass2jax._bass_exec_p.bind(
            *operands,
            out_avals=tuple(out_avals),
            in_names=tuple(all_in_names),
            out_names=tuple(out_names),
            lowering_input_output_aliases=(),
            sim_require_finite=True,
            sim_require_nnan=True,
            nc=nc,
        )
        return tuple(outs)

    n_cores = 8
    devices = jax.devices()[:n_cores]
    mesh = Mesh(np.asarray(devices), ("core",))
    in_specs = (PartitionSpec("core"),) * (n_params + n_outs)
    out_specs = (PartitionSpec("core"),) * n_outs
    sharded = jax.jit(
        shard_map(_body, mesh=mesh, in_specs=in_specs, out_specs=out_specs,
                  check_rep=False),
        donate_argnums=donate, keep_unused=True)

    def run(in_maps):
        per_core = [[np.asarray(m[name]) for name in in_names] for m in in_maps]
        concat_in = [np.concatenate([per_core[c][i] for c in range(n_cores)],
                                    axis=0) for i in range(n_params)]
        concat_zeros = [np.zeros((n_cores * z.shape[0], *z.shape[1:]), z.dtype)
                        for z in zero_outs]
        out_arrs = sharded(*concat_in, *concat_zeros)
        return [
            {name: np.asarray(out_arrs[i]).reshape(n_cores,
                                                   *out_avals[i].shape)[c]
             for i, name in enumerate(out_names)}
            for c in range(n_cores)
        ]

    _CACHE[key] = run
    return run


def _get_bench(reps=1, skip=None):
    """Zero-transfer bench callable: inputs pre-placed on device, outputs
    left on device (block_until_ready only). No donation."""
    if skip is None:
        skip = DEFAULT_SKIP
    key = ("bench", reps, tuple(skip))
    if key in _CACHE:
        return _CACHE[key]
    import jax
    from jax.sharding import Mesh, PartitionSpec, NamedSharding
    from jax.experimental.shard_map import shard_map
    from concourse import bass2jax

    nc = build_nc(reps, skip)
    bass2jax.install_neuronx_cc_hook()
    partition_name = (nc.partition_id_tensor.name
                      if nc.partition_id_tensor else None)
    in_names, out_names, out_avals, zero_outs = [], [], [], []
    for alloc in nc.m.functions[0].allocations:
        if not isinstance(alloc, mybir.MemoryLocationSet):
            continue
        name = alloc.memorylocations[0].name
        if alloc.kind == "ExternalInput":
            if name != partition_name:
                in_names.append(name)
        elif alloc.kind == "ExternalOutput":
            out_names.append(name)
            shape = tuple(alloc.tensor_shape)
            dtype = mybir.dt.np(alloc.dtype)
            out_avals.append(jax.core.ShapedArray(shape, dtype))
            zero_outs.append(np.zeros(shape, dtype))
    n_params = len(in_names)
    all_in_names = list(in_names) + list(out_names)
    if partition_name is not None:
        all_in_names.append(partition_name)

    def _body(*args):
        operands = list(args)
        if partition_name is not None:
            operands.append(bass2jax.partition_id_tensor())
        outs = bass2jax._bass_exec_p.bind(
            *operands,
            out_avals=tuple(out_avals),
            in_names=tuple(all_in_names),
            out_names=tuple(out_names),
            lowering_input_output_aliases=(),
            sim_require_finite=True,
            sim_require_nnan=True,
            nc=nc,
        )
        return tuple(outs)

    n_cores = 8
    devices = jax.devices()[:n_cores]
    mesh = Mesh(np.asarray(devices), ("core",))
    nouts = len(out_names)
    in_specs = (PartitionSpec("core"),) * (n_params + nouts)
    out_specs = (PartitionSpec("core"),) * nouts
    sharded = jax.jit(
        shard_map(_body, mesh=mesh, in_specs=in_specs, out_specs=out_specs,
                  check_rep=False),
        keep_unused=True)
    shard = NamedSharding(mesh, PartitionSpec("core"))

    def make_args(in_maps):
        per_core = [[np.asarray(m[name]) for name in in_names]
                    for m in in_maps]
        concat_in = [np.concatenate([per_core[c][i] for c in range(n_cores)],
                                    axis=0) for i in range(n_params)]
        concat_zeros = [np.zeros((n_cores * z.shape[0], *z.shape[1:]),
                                 z.dtype) for z in zero_outs]
        return [jax.device_put(a, shard) for a in concat_in + concat_zeros]

    def call(dev_args):
        outs = sharded(*dev_args)
        for o in outs:
            o.block_until_ready()
        return outs

    call.sharded = sharded
    result = (make_args, call)
    _CACHE[key] = result
    return result


def _prep_in_maps(x, w_qkv, w_out, skip=None):
    if skip is None:
        skip = DEFAULT_SKIP
    import ml_dtypes
    bf = ml_dtypes.bfloat16
    x = np.asarray(x, dtype=np.float32)
    w_qkv = np.asarray(w_qkv, dtype=np.float32)
    w_out = np.asarray(w_out, dtype=np.float32)
    msk = np.zeros((128, 256), dtype=np.float32)
    msk[:, 0:128] = np.triu(np.full((128, 128), NEG, dtype=np.float32), k=1)
    msk[:, 128:256] = np.eye(128, dtype=np.float32)
    msk = msk.astype(bf)
    in_maps = []
    xts = [np.ascontiguousarray(x[b].T).astype(bf) for b in range(B)]

    for core in range(8):
        b, g = divmod(core, 4)
        cl, ch = 256 * g, 256 * g + 256
        wqkv = np.ascontiguousarray(np.concatenate(
            [w_qkv[:, cl:ch], w_qkv[:, C + cl:C + ch],
             w_qkv[:, 2 * C + cl:2 * C + ch]], axis=1)).astype(bf)
        wo = np.ascontiguousarray(w_out[cl:ch, :]).astype(bf)
        in_maps.append({"xt": xts[b], "wqkv": wqkv, "wo": wo, "msk": msk})
    return in_maps


def kernel(x, w_qkv, w_out):
    run = _get_runner()
    in_maps = _prep_in_maps(x, w_qkv, w_out)
    results = run(in_maps)
    y = np.zeros((B, T, C), dtype=np.float32)
    for core in range(8):
        b = core // 4
        y[b] += results[core]["y"].astype(np.float32)
    return y


if __name__ == "__main__":
    rng = np.random.default_rng(0)
    x = rng.standard_normal((B, T, C)).astype(np.float32)
    w_qkv = (rng.standard_normal((C, 3 * C)) / np.sqrt(C)).astype(np.float32)
    w_out = (rng.standard_normal((C, C)) / np.sqrt(C)).astype(np.float32)
    y = kernel(x=x, w_qkv=w_qkv, w_out=w_out)
    print("kernel ran, y:", y.shape, y.dtype, float(np.abs(y).max()))



# revision 68
# speedup vs baseline: 1.0769x; 1.0175x over previous
"""Trainium2 Bass kernel for multi-head causal attention (v3, bf16+fp8).

Problem (hardcoded): x [2, 2048, 1024] fp32, w_qkv [1024, 3072], w_out
[1024, 1024].
  qkv = x @ w_qkv; per-head causal softmax attention (16 heads, d=64);
  out = attn_out @ w_out.

Sharding: 8 cores = (2 batches) x (4 head-groups of 4 heads).
Each core computes, for its batch b and heads 4g..4g+3 (2 pairs of 2 heads):
  - Q^T, K^T [256, 2048] and V [2048, 256] from x[b]^T (host-pretransposed,
    bf16) via PE, pipelined as filler work between attention groups
  - causal attention on-chip in S^T layout (S in bf16 — fp8 QK noise breaks
    tolerance); diagonal tiles are column-sliced so fully-masked columns are
    never computed, the remaining 128-col triangle zeroed post-exp by a
    gpsimd affine_select; rowsum via a ones-column in the AV stationary
  - AV for q-chunks j>=1 runs in fp8e4 DoubleRow (2 k-tiles per matmul at
    0.5 cyc/col; exp writes p2t as fp8 with a -4 logit shift so the
    numerator can't overflow e4m3's 448 max — the shift cancels in the
    normalize).  Chunk 0 keeps the bf16 path: with <512 keys the fp8 P/V
    quantization noise doesn't average out.
  - per-head group ordering (h2-outer): each head's AV bank drains (at-copy
    on ACT, reciprocal on DVE, DRAM-hop broadcast DMA) while the other
    head's attention still runs; normalization mul on DVE once both
    broadcast halves land
  - out-projection + drains pipelined against the next block; attention
    tiles parity-alternate between reps so a benched rep's projections
    overlap the previous rep's tail
Host gathers: y[b] = sum_g y_part[4b+g] (partials stored bf16).
"""
import numpy as np

import concourse.bass as bass
from concourse import bacc
import concourse.mybir as mybir
import concourse.tile as tile

F32 = mybir.dt.float32
F32R = mybir.dt.float32r
BF16 = mybir.dt.bfloat16
FP8 = mybir.dt.float8e4
AF = mybir.ActivationFunctionType
DR = mybir.MatmulPerfMode.DoubleRow

B, T, C = 2, 2048, 1024
H_TOT, D = 16, 64
HL = 4             # heads per core
DL = HL * D        # 256 local channels
NJ = 4             # q-chunks of 512
NKT = 16           # k-tiles of 128
NCT = 8            # c-tiles of 128 (contraction over C)
SM_SCALE = 1.0 / np.sqrt(D)
NEG = -30000.0

_CACHE = {}

# variant used by kernel() and by test.py's default bench:
#   fp8   - AV via fp8e4 DoubleRow for chunks j>=1 (chunk 0 stays bf16)
#   dexp1 - single-span exp on diagonal groups
#   xrep  - parity-alternated attention tiles (cross-rep overlap)
#   atact - drain PSUM->SBUF copies on ACT (frees the DVE release path)
#   h2o   - per-head group ordering with early per-head drains
#   pend2 - AV emission lags S/exp by 2 groups
#   dr3   - broadcast the raw rowsum; one post-DMA reciprocal on the
#           broadcast result keeps DVE off the PSUM-release path
DEFAULT_SKIP = ("fp8", "dexp1", "xrep", "atact", "h2o", "pend2", "dr3")


def build_nc(reps=1, skip=()):
    nc = bacc.Bacc("TRN2", target_bir_lowering=False)
    xt = nc.dram_tensor("xt", [C, T], BF16, kind="ExternalInput")
    wqkv = nc.dram_tensor("wqkv", [C, 3 * DL], BF16, kind="ExternalInput")
    wo = nc.dram_tensor("wo", [DL, C], BF16, kind="ExternalInput")
    msk = nc.dram_tensor("msk", [128, 256], BF16, kind="ExternalInput")
    y = nc.dram_tensor("y", [T, C], BF16, kind="ExternalOutput")
    if "rcout" in skip:
        rcdbg = nc.dram_tensor("rcdbg", [16, 512], F32, kind="ExternalOutput")

    with tile.TileContext(nc) as tc:
      for _rep in range(reps):
        with tc.tile_pool(name="persist", bufs=1) as persist, \
             tc.tile_pool(name="dram", bufs=1, space="DRAM") as drampool, \
             tc.tile_pool(name="rcp", bufs=6) as rcp, \
             tc.tile_pool(name="bcp", bufs=3) as bcp, \
             tc.tile_pool(name="pexp", bufs=6) as pexp, \
             tc.tile_pool(name="ysbp", bufs=4) as ysbp, \
             tc.tile_pool(name="s2p",
                          bufs=1 if "bigexp" in skip else 2,
                          space="PSUM") as s2p, \
             tc.tile_pool(name="otp", bufs=1, space="PSUM") as otp, \
             tc.tile_pool(name="ppp", bufs=2, space="PSUM") as ppp:
            use8 = "fp8" in skip
            # cross-rep parity: alternate the attention tiles between reps
            # so rep N+1's projections aren't WAR-blocked on rep N's last
            # S/AV reads (benched steady-state overlaps rep tails)
            par = _rep % 2 if "xrep" in skip else 0
            qk_tiles = [persist.tile([128, T], BF16, tag=f"qk{m}_{par}",
                                     name=f"qk{m}_{par}") for m in range(4)]
            qt_t, kt_t = qk_tiles[0:2], qk_tiles[2:4]
            # S stays bf16 (fp8 QK noise breaks tolerance); AV for chunks
            # j>=1 uses fp8 DoubleRow (chunk 0 has too few keys to average
            # out fp8 P/V noise, so it keeps the bf16 path and only needs
            # bf16 V for k-tiles 0..3).
            nvsb = 4 if use8 else NKT
            v_sb = [persist.tile([128, HL, D + 1], BF16, tag=f"v{t}_{par}",
                                 name=f"v{t}_{par}") for t in range(nvsb)]
            if use8:
                # vp8 per k-tile pair [128p, 2 ktile, HL, 128] (col D = ones
                # rowsum, col D+1 pad: stationary 66 wide; per-head width
                # padded to 128 so the DoubleRow stationary's k-subtile
                # stride is 512B, an ISA restriction)
                vp8 = [persist.tile([128, 2, HL, 128], FP8,
                                    tag=f"vp{t}_{par}",
                                    name=f"vp{t}_{par}")
                       for t in range(NKT // 2)]
                ebias = persist.tile([128, 1], F32, tag="eb", name="eb")
                nc.vector.memset(ebias[:], -4.0)
            at_t = [persist.tile([128, T], BF16, tag=f"at{p}_{par}",
                                 name=f"at{p}_{par}") for p in range(2)]
            xt_sb = [persist.tile([128, T], BF16, tag=f"xt{c}", name=f"xt{c}")
                     for c in range(NCT)]
            wqkv_sb = [persist.tile([128, 3 * DL], BF16, tag=f"wq{c}",
                                    name=f"wq{c}") for c in range(NCT)]
            wo_sb = [persist.tile([128, C], BF16, tag=f"wo{i}", name=f"wo{i}")
                     for i in range(2)]
            msk_sb = persist.tile([128, 256], BF16, tag="msk", name="msk")
            otd = 66 if use8 else 65
            ot_ps = [otp.tile([otd, 512], F32, tag=f"ot{h2}", name=f"ot{h2}")
                     for h2 in range(2)]
            if "rcout" in skip:
                rc_dram = rcdbg[:, :]
            else:
                rc_dram = drampool.tile([16, 512], F32)

            if "probe" in skip:
                pr = persist.tile([128, 2, 512], FP8, tag="pr8", name="pr8")
                pr2 = persist.tile([128, 2, 65], FP8, tag="pr8b", name="pr8b")
                prq = persist.tile([128, 2, 512], FP8, tag="pr8c", name="pr8c")
                prps = ppp.tile([128, 512], F32, tag="pp", name="pp")
                nc.tensor.matmul(prps[:], wqkv_sb[0][:, 0:128],
                                 xt_sb[0][:, 0:512], start=True, stop=True)
                # (a) DVE f32->fp8 cast from PSUM
                nc.vector.tensor_copy(pr[:, 0, :], prps[:])
                # (b) ACT exp with fp8 out
                nc.scalar.activation(pr[:, 1, :], prps[:], AF.Exp, scale=0.01)
                # (c) affine_select on fp8
                nc.gpsimd.affine_select(
                    out=pr[:, 0, 0:128], in_=pr[:, 0, 0:128],
                    compare_op=mybir.AluOpType.is_ge, fill=0.0, base=0,
                    pattern=[[1, 128]], channel_multiplier=-1)
                # (e) fp8 memset
                nc.vector.memset(pr2[:, :, 64:65], 1.0)
                nc.vector.tensor_copy(pr2[:, 0, 0:64], prps[:, 0:64])
                nc.vector.tensor_copy(prq[:, :, 0:256],
                                      prps[:].rearrange("p (k q) -> p k q",
                                                        k=2))
                ps8 = ppp.tile([128, 512], F32, tag="pp", name="pp")
                if "pa" in skip:   # AV-style DoubleRow, M=65 (odd)
                    nc.tensor.matmul(ps8[0:65, :], pr2[:, :, :], pr[:, :, :],
                                     start=True, stop=True, perf_mode=DR)
                if "pb" in skip:   # AV-style DoubleRow, M=64
                    nc.tensor.matmul(ps8[0:64, :], pr2[:, :, 0:64],
                                     pr[:, :, :], start=True, stop=True,
                                     perf_mode=DR)
                if "pc" in skip:   # AV-style DoubleRow, M=66 via pr tile
                    nc.tensor.matmul(ps8[0:66, :], pr[:, :, 0:66],
                                     pr[:, :, :], start=True, stop=True,
                                     perf_mode=DR)
                if "pd" in skip:   # S-style 32-row DoubleRow at base 32
                    nc.tensor.matmul(ps8[:, 0:512], prq[32:64, :, 0:128],
                                     prq[32:64, :, :],
                                     start=True, stop=True, perf_mode=DR)
                if "pe2" in skip:  # mixed accumulation group DR + plain fp8
                    nc.tensor.matmul(ps8[0:66, :], pr[:, :, 0:66],
                                     pr[:, :, :], start=True, stop=False,
                                     perf_mode=DR)
                    nc.tensor.matmul(ps8[0:66, 128:512], pr[:, 0, 0:66],
                                     pr[:, 0, 128:512], start=False,
                                     stop=True)
                nc.vector.tensor_copy(at_t[0][:, 0:512], ps8[:])

            # ---- loads ----
            # xt column-split: chunk-0 columns first so projections for j=0
            # can start before the bulk of x arrives.
            for c in range(NCT):
                eng = nc.sync if c % 2 == 0 else nc.scalar
                eng.dma_start(out=xt_sb[c][:, 0:512],
                              in_=xt[128 * c:128 * (c + 1), 0:512])
                (nc.scalar if c % 2 == 0 else nc.sync).dma_start(
                    out=wqkv_sb[c][:], in_=wqkv[128 * c:128 * (c + 1), :])
            for c in range(NCT):
                (nc.sync if c % 2 == 0 else nc.scalar).dma_start(
                    out=xt_sb[c][:, 512:T],
                    in_=xt[128 * c:128 * (c + 1), 512:T])
            for i in range(2):
                nc.sync.dma_start(out=wo_sb[i][:],
                                  in_=wo[128 * i:128 * (i + 1), :])
            nc.scalar.dma_start(out=msk_sb[:], in_=msk[:, :])
            sum0 = "sum0" in skip
            vco = 1 if sum0 else 0     # v data column offset in stationaries
            if use8:
                for tp in range(NKT // 2):
                    if sum0:
                        nc.vector.memset(vp8[tp][:, :, :, 0:1], 1.0)
                        nc.vector.memset(vp8[tp][:, :, :, 65:66], 1.0)
                    else:
                        nc.vector.memset(vp8[tp][:, :, :, D:D + 2], 1.0)
            for t in range(nvsb):
                if sum0:
                    nc.vector.memset(v_sb[t][:, :, 0:1], 1.0)
                else:
                    nc.vector.memset(v_sb[t][:, :, D:D + 1], 1.0)

            def qk_chunk(m, j, defer=False):
                ps = ppp.tile([128, 512], F32, tag="pp", name="pp")
                for c in range(NCT):
                    nc.tensor.matmul(
                        ps[:],
                        wqkv_sb[c][:, 128 * m:128 * (m + 1)],
                        xt_sb[c][:, 512 * j:512 * (j + 1)],
                        start=(c == 0), stop=(c == NCT - 1))

                def copy():
                    nc.vector.tensor_copy(
                        qk_tiles[m][:, 512 * j:512 * (j + 1)], ps[:])
                if defer:
                    return copy
                copy()

            def v_tile(t, defer=False):
                ps = ppp.tile([128, 512], F32, tag="pp", name="pp")
                for c in range(NCT):
                    nc.tensor.matmul(
                        ps[:, 0:DL],
                        xt_sb[c][:, 128 * t:128 * (t + 1)],
                        wqkv_sb[c][:, 2 * DL:3 * DL],
                        start=(c == 0), stop=(c == NCT - 1))

                def copy():
                    if use8:
                        nc.vector.tensor_copy(
                            vp8[t // 2][:, t % 2, :, vco:vco + D],
                            ps[:, 0:DL].rearrange("p (h d) -> p h d", h=HL))
                        if t < nvsb:
                            nc.vector.tensor_copy(
                                v_sb[t][:, :, vco:vco + D],
                                ps[:, 0:DL].rearrange("p (h d) -> p h d",
                                                      h=HL))
                    else:
                        nc.vector.tensor_copy(
                            v_sb[t][:, :, vco:vco + D],
                            ps[:, 0:DL].rearrange("p (h d) -> p h d", h=HL))
                if defer:
                    return copy
                copy()

            def s_group(pair, j, ktg, h2):
                """Emit S^T matmuls (+pre-exp causal mask) for one s2 group
                (2 k-tiles); returns the s2 tile and exp metadata."""
                s2 = s2p.tile([128, 1024], F32, tag="s", name="s")
                diag = (ktg >= 2 * j)
                segs = []
                for kk in range(2):
                    ktt = 2 * ktg + kk
                    col0 = 512 * kk
                    q0 = 0 if not diag else 128 * (ktt - 4 * j)
                    qt, kt = qt_t[pair], kt_t[pair]
                    base = 64 * h2
                    nc.tensor.matmul(
                        s2[:, col0 + q0:col0 + 512],
                        kt[base:base + 64, 128 * ktt:128 * (ktt + 1)],
                        qt[base:base + 64, 512 * j + q0:512 * (j + 1)],
                        start=True, stop=True)
                    segs.append((col0 + q0, 512 - q0))
                return s2, diag, segs

            def exp_group(s2, diag, segs, p8):
                p2t = pexp.tile([128, 1024], FP8 if p8 else BF16,
                                tag="p8" if p8 else "p",
                                name="p8" if p8 else "p")
                if "noexp" in skip:
                    return p2t
                # fp8e4 has no inf and max 448: shift logits down so the
                # softmax numerator never overflows (cancels in normalize)
                eb = ebias[:, :] if p8 else 0.0
                nrep = 2 if "2xexp" in skip else 1
                if not diag:
                    for _ in range(nrep):
                        nc.scalar.activation(p2t[:], s2[:], AF.Exp,
                                             scale=float(SM_SCALE), bias=eb)
                elif "dexp1" in skip:
                    # single span including the dead gap between segments
                    col_lo = segs[0][0]
                    nc.scalar.activation(p2t[:, col_lo:1024],
                                         s2[:, col_lo:1024], AF.Exp,
                                         scale=float(SM_SCALE), bias=eb)
                else:
                    for col0, w in segs:
                        for _ in range(nrep):
                            nc.scalar.activation(p2t[:, col0:col0 + w],
                                                 s2[:, col0:col0 + w], AF.Exp,
                                                 scale=float(SM_SCALE),
                                                 bias=eb)
                return p2t

            def av_group(pair, j, ktg, h2, p2t, diag, segs):
                """ot bank group: start=True only on the block's first matmul
                (clears the bank), stop=True only on its last (r=3 part A)."""
                h = 2 * pair + h2
                ot = ot_ps[h2]
                p8 = use8 and j >= 1
                if p8 and not diag:
                    # one DoubleRow matmul covers both k-tiles of the group
                    nc.tensor.matmul(
                        ot[:, 0:512], vp8[ktg][:, :, h, 0:D + 2],
                        p2t[:].rearrange("p (k q) -> p k q", k=2),
                        start=(ktg == 0), stop=False, perf_mode=DR)
                    return
                for kk in range(2):
                    ktt = 2 * ktg + kk
                    col0 = 512 * kk
                    if p8:
                        vv = vp8[ktg][:, kk, h, 0:D + 2]
                        ot = ot_ps[h2]
                    else:
                        vv = v_sb[ktt][:, h, 0:D + 1]
                        ot = ot_ps[h2][0:65]
                    if not diag:
                        nc.tensor.matmul(ot[:, 0:512], vv,
                                         p2t[:, col0:col0 + 512],
                                         start=(ktt == 0), stop=False)
                        if "2xav" in skip:
                            nc.tensor.matmul(ot[:, 0:512], vv,
                                             p2t[:, col0:col0 + 512],
                                             start=False, stop=False)
                    else:
                        r = ktt - 4 * j
                        q0 = 128 * r
                        if "nomask" not in skip:
                            nc.gpsimd.affine_select(
                                out=p2t[:, col0 + q0:col0 + q0 + 128],
                                in_=p2t[:, col0 + q0:col0 + q0 + 128],
                                compare_op=mybir.AluOpType.is_ge,
                                fill=0.0, base=0,
                                pattern=[[1, 128]],
                                channel_multiplier=-1)
                        first = (j == 0 and r == 0)
                        nc.tensor.matmul(ot[:, q0:512], vv,
                                         p2t[:, col0 + q0:col0 + 512],
                                         start=first, stop=(r == 3))

            def att_block(pair, j, fillers=()):
                """Software-pipelined S -> exp -> AV over all groups.
                `fillers` are independent emission closures (projections for
                the next chunk) slotted between groups to keep PE fed while
                the exp chain runs."""
                fillers = list(fillers)
                deferred = []
                fdef = "fdefer" in skip
                lag = 3 if "pend3" in skip else (2 if "pend2" in skip else 1)
                if "bigexp" in skip:
                    # super-groups of 4 k-tiles over one 4-bank s4 tile;
                    # a single 2048-wide exp per super-group (h2-outer)
                    bc = bcp.tile([128, 512], F32, tag="bc", name="bc")
                    nsg = j + 1
                    p8 = use8 and j >= 1

                    def av_sg(h2, sg, p4, diag_sg):
                        h = 2 * pair + h2
                        for kk in range(4):
                            ktt = 4 * sg + kk
                            col0 = 512 * kk
                            if diag_sg:
                                r = ktt - 4 * j
                                q0 = 128 * r
                                nc.gpsimd.affine_select(
                                    out=p4[:, col0 + q0:col0 + q0 + 128],
                                    in_=p4[:, col0 + q0:col0 + q0 + 128],
                                    compare_op=mybir.AluOpType.is_ge,
                                    fill=0.0, base=0, pattern=[[1, 128]],
                                    channel_multiplier=-1)
                                if p8:
                                    vv = vp8[2 * sg + kk // 2][
                                        :, kk % 2, h, 0:D + 2]
                                    oto = ot_ps[h2]
                                else:
                                    vv = v_sb[ktt][:, h, 0:D + 1]
                                    oto = ot_ps[h2][0:65]
                                first = (j == 0 and r == 0)
                                nc.tensor.matmul(
                                    oto[:, q0:512], vv,
                                    p4[:, col0 + q0:col0 + 512],
                                    start=first, stop=(r == 3))
                            elif p8:
                                if kk % 2 == 1:
                                    continue
                                ktg = 2 * sg + kk // 2
                                nc.tensor.matmul(
                                    ot_ps[h2][:, 0:512],
                                    vp8[ktg][:, :, h, 0:D + 2],
                                    p4[:, col0:col0 + 1024].rearrange(
                                        "p (k q) -> p k q", k=2),
                                    start=(ktg == 0), stop=False,
                                    perf_mode=DR)
                            else:
                                nc.tensor.matmul(
                                    ot_ps[h2][0:65, 0:512],
                                    v_sb[ktt][:, h, 0:D + 1],
                                    p4[:, col0:col0 + 512],
                                    start=(ktt == 0), stop=False)

                    for h2 in range(2):
                        pend = []
                        for sg in range(nsg):
                            diag_sg = (sg == j)
                            s4 = s2p.tile([128, 2048], F32, tag="s",
                                          name="s")
                            for kk in range(4):
                                ktt = 4 * sg + kk
                                col0 = 512 * kk
                                r = ktt - 4 * j
                                q0 = 128 * r if (diag_sg and r > 0) else 0
                                qt, kt = qt_t[pair], kt_t[pair]
                                base = 64 * h2
                                nc.tensor.matmul(
                                    s4[:, col0 + q0:col0 + 512],
                                    kt[base:base + 64,
                                       128 * ktt:128 * (ktt + 1)],
                                    qt[base:base + 64,
                                       512 * j + q0:512 * (j + 1)],
                                    start=True, stop=True)
                            p4 = pexp.tile([128, 2048],
                                           FP8 if p8 else BF16,
                                           tag="p8" if p8 else "p",
                                           name="p8" if p8 else "p")
                            eb = ebias[:, :] if p8 else 0.0
                            nc.scalar.activation(p4[:], s4[:], AF.Exp,
                                                 scale=float(SM_SCALE),
                                                 bias=eb)
                            pend.append((sg, p4, diag_sg))
                            if fillers:
                                d = fillers.pop(0)(False)
                                if d is not None:
                                    deferred.append(d)
                            if len(pend) > 1:
                                g = pend.pop(0)
                                av_sg(h2, g[0], g[1], g[2])
                        for g in pend:
                            av_sg(h2, g[0], g[1], g[2])
                        drain_h2(pair, j, h2, bc)
                    drain_mul(pair, j, bc)
                    for f in fillers:
                        d = f(False)
                        if d is not None:
                            deferred.append(d)
                    return deferred
                if "h2o" in skip:
                    # h2-outer: finish one head's groups (and its drain)
                    # before the other's, so each ot bank is released and
                    # normalized mid-block instead of both at block end
                    bc = bcp.tile([128, 512], F32, tag="bc", name="bc")
                    ng = 2 * (j + 1)
                    for h2 in range(2):
                        pend = []
                        for ktg in range(ng):
                            s2, diag, segs = s_group(pair, j, ktg, h2)
                            p2t = exp_group(s2, diag, segs, use8 and j >= 1)
                            pend.append((ktg, h2, p2t, diag, segs))
                            if fillers:
                                late = fdef and (h2 == 1 and ktg >= ng - 3)
                                d = fillers.pop(0)(late)
                                if d is not None:
                                    deferred.append(d)
                            if len(pend) > lag:
                                g = pend.pop(0)
                                av_group(pair, j, g[0], g[1], g[2], g[3],
                                         g[4])
                        for g in pend:
                            av_group(pair, j, g[0], g[1], g[2], g[3], g[4])
                        drain_h2(pair, j, h2, bc)
                    drain_mul(pair, j, bc)
                    for f in fillers:
                        d = f(fdef)
                        if d is not None:
                            deferred.append(d)
                    return deferred
                glist = [(ktg, h2) for ktg in range(2 * (j + 1))
                         for h2 in range(2)]
                pend = []   # (ktg, h2, p2t, diag, segs) awaiting AV emission
                for i, (ktg, h2) in enumerate(glist):
                    s2, diag, segs = s_group(pair, j, ktg, h2)
                    p2t = exp_group(s2, diag, segs, use8 and j >= 1)
                    pend.append((ktg, h2, p2t, diag, segs))
                    if fillers:
                        late = fdef and i >= len(glist) - 3
                        d = fillers.pop(0)(late)
                        if d is not None:
                            deferred.append(d)
                    if i >= lag:
                        g = pend.pop(0)
                        av_group(pair, j, g[0], g[1], g[2], g[3], g[4])
                for g in pend:
                    av_group(pair, j, g[0], g[1], g[2], g[3], g[4])
                for f in fillers:
                    d = f(fdef)
                    if d is not None:
                        deferred.append(d)
                return deferred

            def drain_h2(pair, j, h2, bc):
                """Per-head drain: at-copy + recip + broadcast into bc half."""
                cp = (nc.scalar.copy if "atact" in skip
                      else nc.vector.tensor_copy)
                cp(at_t[pair][64 * h2:64 * h2 + 64,
                              512 * j:512 * (j + 1)],
                   ot_ps[h2][0:64, :])
                rc = rcp.tile([65, 512], F32, tag="rc", name="rc")
                if "dr3" in skip:
                    # broadcast the RAW rowsum; reciprocal happens once on
                    # the broadcast result (drain_mul), so the DVE is fully
                    # off the PSUM-release path
                    if "atact" in skip:
                        nc.scalar.copy(out=rc[64:65, :],
                                       in_=ot_ps[h2][64:65, :])
                    else:
                        nc.vector.tensor_copy(rc[64:65, :],
                                              ot_ps[h2][64:65, :])
                else:
                    nc.vector.reciprocal(out=rc[64:65, :],
                                         in_=ot_ps[h2][64:65, :])
                idx = 4 * j + 2 * pair + h2
                dq = nc.gpsimd if "rcpool" in skip else nc.sync
                dq.dma_start(out=rc_dram[idx:idx + 1, :], in_=rc[64:65, :])
                seg = rc_dram[idx:idx + 1, :]
                bsrc = bass.AP(tensor=seg.tensor, offset=seg.offset,
                               ap=[[0, 64]] + list(seg.ap))
                dq.dma_start(
                    out=bc[64 * h2:64 * h2 + 64, :].rearrange(
                        "p (a b) -> p a b", a=1),
                    in_=bsrc)

            def drain_mul(pair, j, bc):
                if "dr3" in skip:
                    nc.vector.reciprocal(out=bc[:], in_=bc[:])
                nc.vector.tensor_mul(
                    at_t[pair][:, 512 * j:512 * (j + 1)],
                    at_t[pair][:, 512 * j:512 * (j + 1)],
                    bc[:])

            def drain_block(pair, j):
                """Copy AV out to at_t, recip rowsums, broadcast, normalize."""
                if "nodrain" in skip or "h2o" in skip:
                    return
                bc = bcp.tile([128, 512], F32, tag="bc", name="bc")
                if "dr2" in skip:
                    # store raw rowsum rows straight from PSUM, broadcast
                    # both halves, one recip over [128,512], then normalize
                    for h2 in range(2):
                        cp = (nc.scalar.copy if "atact" in skip
                              else nc.vector.tensor_copy)
                        cp(at_t[pair][64 * h2:64 * h2 + 64,
                                      512 * j:512 * (j + 1)],
                           ot_ps[h2][0:64, :])
                        idx = 4 * j + 2 * pair + h2
                        nc.sync.dma_start(out=rc_dram[idx:idx + 1, :],
                                          in_=ot_ps[h2][64:65, :])
                        seg = rc_dram[idx:idx + 1, :]
                        bsrc = bass.AP(tensor=seg.tensor, offset=seg.offset,
                                       ap=[[0, 64]] + list(seg.ap))
                        nc.sync.dma_start(
                            out=bc[64 * h2:64 * h2 + 64, :].rearrange(
                                "p (a b) -> p a b", a=1),
                            in_=bsrc)
                    nc.vector.reciprocal(out=bc[:], in_=bc[:])
                    nc.vector.tensor_mul(
                        at_t[pair][:, 512 * j:512 * (j + 1)],
                        at_t[pair][:, 512 * j:512 * (j + 1)],
                        bc[:])
                    return
                d0 = 1 if sum0 else 0
                sr = 0 if sum0 else 64   # rowsum partition row in ot
                for h2 in range(2):
                    cp = (nc.scalar.copy if "atact" in skip
                          else nc.vector.tensor_copy)
                    cp(at_t[pair][64 * h2:64 * h2 + 64,
                                  512 * j:512 * (j + 1)],
                       ot_ps[h2][d0:d0 + 64, :])
                    rc = rcp.tile([65, 512], F32, tag="rc", name="rc")
                    if "pbc" in skip and sum0:
                        # partition_broadcast broadcasts partition 0, so it
                        # needs the rowsum (and its reciprocal) on row 0
                        nc.vector.reciprocal(
                            out=rc[0:1, :], in_=ot_ps[h2][0:1, :])
                        nc.gpsimd.partition_broadcast(
                            bc[64 * h2:64 * h2 + 64, :], rc[0:1, :])
                    elif "sbbc" in skip:
                        # broadcast straight from the SBUF rc row with a
                        # partition-stride-0 DMA source (no DRAM hop)
                        nc.vector.reciprocal(
                            out=rc[sr:sr + 1, :],
                            in_=ot_ps[h2][sr:sr + 1, :])
                        src = rc[sr:sr + 1, :]
                        bsrc = bass.AP(tensor=src.tensor, offset=src.offset,
                                       ap=[[0, 64]] + list(src.ap))
                        nc.sync.dma_start(
                            out=bc[64 * h2:64 * h2 + 64, :].rearrange(
                                "p (a b) -> p a b", a=1),
                            in_=bsrc)
                    else:
                        for _ in range(2 if "2xrecip" in skip else 1):
                            nc.vector.reciprocal(
                                out=rc[sr:sr + 1, :],
                                in_=ot_ps[h2][sr:sr + 1, :])
                        idx = 4 * j + 2 * pair + h2
                        dq = nc.gpsimd if "rcpool" in skip else nc.sync
                        dq.dma_start(out=rc_dram[idx:idx + 1, :],
                                     in_=rc[sr:sr + 1, :])
                        seg = rc_dram[idx:idx + 1, :]
                        bsrc = bass.AP(tensor=seg.tensor, offset=seg.offset,
                                       ap=[[0, 64]] + list(seg.ap))
                        dq.dma_start(
                            out=bc[64 * h2:64 * h2 + 64, :].rearrange(
                                "p (a b) -> p a b", a=1),
                            in_=bsrc)
                mul = (nc.gpsimd.tensor_mul if "mulpool" in skip
                       else nc.vector.tensor_mul)
                mul(at_t[pair][:, 512 * j:512 * (j + 1)],
                    at_t[pair][:, 512 * j:512 * (j + 1)],
                    bc[:])

            def outp_unit(j, t, oc):
                        yps = ppp.tile([128, 512], F32, tag="pp", name="pp")
                        for i in range(2):
                            nc.tensor.matmul(
                                yps[:],
                                at_t[i][:, 128 * t:128 * (t + 1)],
                                wo_sb[i][:, 512 * oc:512 * (oc + 1)],
                                start=(i == 0), stop=(i == 1))
                        ysb = ysbp.tile([128, 512], BF16, tag="ysb",
                                        name="ysb")
                        if j == NJ - 1 and oc == 1:
                            # tail: drain on ACT in parallel with DVE
                            nc.scalar.copy(out=ysb[:], in_=yps[:])
                        elif "ysbact" in skip:
                            nc.scalar.copy(out=ysb[:], in_=yps[:])
                        elif "ysbpool" in skip:
                            nc.gpsimd.tensor_copy(ysb[:], yps[:])
                        else:
                            for _ in range(2 if "2xcopy" in skip else 1):
                                nc.vector.tensor_copy(ysb[:], yps[:])
                        (nc.sync if oc == 0 else nc.gpsimd).dma_start(
                            out=y[128 * t:128 * (t + 1),
                                  512 * oc:512 * (oc + 1)],
                            in_=ysb[:])

            def outp(j):
                if "nooutp" in skip:
                    return
                for t in range(4 * j, 4 * j + 4):
                    for oc in range(2):
                        outp_unit(j, t, oc)

            # chunk 0's projections up front; later chunks' projections are
            # slotted between attention groups as PE filler work.
            for m in range(4):
                qk_chunk(m, 0)
            for t in range(4):
                v_tile(t)
            for j in range(NJ):
                if j + 1 < NJ:
                    items = ([(lambda late, m=m: qk_chunk(m, j + 1, late))
                              for m in range(4)]
                             + [(lambda late, t=t: v_tile(t, late))
                                for t in range(4 * j + 4, 4 * j + 8)])
                else:
                    items = []
                if "outpfill" in skip:
                    # out-projection of the previous chunk rides along as
                    # PE filler work inside the attention blocks, keeping
                    # ACT fed with exps while PE does projections.
                    oitems = ([(lambda late, t=t, oc=oc, jj=j - 1:
                                outp_unit(jj, t, oc))
                               for t in range(4 * (j - 1), 4 * (j - 1) + 4)
                               for oc in range(2)] if j > 0 else [])
                    both = []
                    na, nb = len(items), len(oitems)
                    for i in range(max(na, nb)):
                        if i < na:
                            both.append(items[i])
                        if i < nb:
                            both.append(oitems[i])
                    half = (len(both) + 1) // 2
                    dd = att_block(0, j, fillers=both[:half])
                    drain_block(0, j)
                    for f in dd:
                        f()
                    dd = att_block(1, j, fillers=both[half:])
                    drain_block(1, j)
                    for f in dd:
                        f()
                else:
                    half = (len(items) + 1) // 2
                    dd = att_block(0, j, fillers=items[:half])
                    drain_block(0, j)
                    for f in dd:
                        f()
                    if j > 0:
                        outp(j - 1)
                    dd = att_block(1, j, fillers=items[half:])
                    drain_block(1, j)
                    for f in dd:
                        f()
            outp(NJ - 1)
    nc.compile()
    return nc


def _get_runner(reps=1, skip=None):
    """Compile once; return a callable(in_maps) -> list of per-core out dicts."""
    if skip is None:
        skip = DEFAULT_SKIP
    key = ("runner", reps, tuple(skip))
    if key in _CACHE:
        return _CACHE[key]
    import jax
    import jax.numpy as jnp
    from jax.sharding import Mesh, PartitionSpec
    from jax.experimental.shard_map import shard_map
    from concourse import bass2jax

    nc = build_nc(reps, skip)
    bass2jax.install_neuronx_cc_hook()

    partition_name = (nc.partition_id_tensor.name
                      if nc.partition_id_tensor else None)
    in_names, out_names, out_avals, zero_outs = [], [], [], []
    for alloc in nc.m.functions[0].allocations:
        if not isinstance(alloc, mybir.MemoryLocationSet):
            continue
        name = alloc.memorylocations[0].name
        if alloc.kind == "ExternalInput":
            if name != partition_name:
                in_names.append(name)
        elif alloc.kind == "ExternalOutput":
            out_names.append(name)
            shape = tuple(alloc.tensor_shape)
            dtype = mybir.dt.np(alloc.dtype)
            out_avals.append(jax.core.ShapedArray(shape, dtype))
            zero_outs.append(np.zeros(shape, dtype))
    n_params = len(in_names)
    n_outs = len(out_avals)
    all_in_names = list(in_names) + list(out_names)
    if partition_name is not None:
        all_in_names.append(partition_name)
    donate = tuple(range(n_params, n_params + n_outs))

    def _body(*args):
        operands = list(args)
        if partition_name is not None:
            operands.append(bass2jax.partition_id_tensor())
        outs = bass2jax._bass_exec_p.bind(
            *operands,
            out_avals=tuple(out_avals),
            in_names=tuple(all_in_names),
            out_names=tuple(out_names),
            lowering_input_output_aliases=(),
            sim_require_finite=True,
            sim_require_nnan=True,
            nc=nc,
        )
        return tuple(outs)

    n_cores = 8
    devices = jax.devices()[:n_cores]
    mesh = Mesh(np.asarray(devices), ("core",))
    in_specs = (PartitionSpec("core"),) * (n_params + n_outs)
    out_specs = (PartitionSpec("core"),) * n_outs
    sharded = jax.jit(
        shard_map(_body, mesh=mesh, in_specs=in_specs, out_specs=out_specs,
                  check_rep=False),
        donate_argnums=donate, keep_unused=True)

    def run(in_maps):
        per_core = [[np.asarray(m[name]) for name in in_names] for m in in_maps]
        concat_in = [np.concatenate([per_core[c][i] for c in range(n_cores)],
                                    axis=0) for i in range(n_params)]
        concat_zeros = [np.zeros((n_cores * z.shape[0], *z.shape[1:]), z.dtype)
                        for z in zero_outs]
        out_arrs = sharded(*concat_in, *concat_zeros)
        return [
            {name: np.asarray(out_arrs[i]).reshape(n_cores,
                                                   *out_avals[i].shape)[c]
             for i, name in enumerate(out_names)}
            for c in range(n_cores)
        ]

    _CACHE[key] = run
    return run


def _get_bench(reps=1, skip=None):
    """Zero-transfer bench callable: inputs pre-placed on device, outputs
    left on device (block_until_ready only). No donation."""
    if skip is None:
        skip = DEFAULT_SKIP
    key = ("bench", reps, tuple(skip))
    if key in _CACHE:
        return _CACHE[key]
    import jax
    from jax.sharding import Mesh, PartitionSpec, NamedSharding
    from jax.experimental.shard_map import shard_map
    from concourse import bass2jax

    nc = build_nc(reps, skip)
    bass2jax.install_neuronx_cc_hook()
    partition_name = (nc.partition_id_tensor.name
                      if nc.partition_id_tensor else None)
    in_names, out_names, out_avals, zero_outs = [], [], [], []
    for alloc in nc.m.functions[0].allocations:
        if not isinstance(alloc, mybir.MemoryLocationSet):
            continue
        name = alloc.memorylocations[0].name
        if alloc.kind == "ExternalInput":
            if name != partition_name:
                in_names.append(name)
        elif alloc.kind == "ExternalOutput":
            out_names.append(name)
            shape = tuple(alloc.tensor_shape)
            dtype = mybir.dt.np(alloc.dtype)
            out_avals.append(jax.core.ShapedArray(shape, dtype))
            zero_outs.append(np.zeros(shape, dtype))
    n_params = len(in_names)
    all_in_names = list(in_names) + list(out_names)
    if partition_name is not None:
        all_in_names.append(partition_name)

    def _body(*args):
        operands = list(args)
        if partition_name is not None:
            operands.append(bass2jax.partition_id_tensor())
        outs = bass2jax._bass_exec_p.bind(
            *operands,
            out_avals=tuple(out_avals),
            in_names=tuple(all_in_names),
            out_names=tuple(out_names),
            lowering_input_output_aliases=(),
            sim_require_finite=True,
            sim_require_nnan=True,
            nc=nc,
        )
        return tuple(outs)

    n_cores = 8
    devices = jax.devices()[:n_cores]
    mesh = Mesh(np.asarray(devices), ("core",))
    nouts = len(out_names)
    in_specs = (PartitionSpec("core"),) * (n_params + nouts)
    out_specs = (PartitionSpec("core"),) * nouts
    sharded = jax.jit(
        shard_map(_body, mesh=mesh, in_specs=in_specs, out_specs=out_specs,
                  check_rep=False),
        keep_unused=True)
    shard = NamedSharding(mesh, PartitionSpec("core"))

    def make_args(in_maps):
        per_core = [[np.asarray(m[name]) for name in in_names]
                    for m in in_maps]
        concat_in = [np.concatenate([per_core[c][i] for c in range(n_cores)],
                                    axis=0) for i in range(n_params)]
        concat_zeros = [np.zeros((n_cores * z.shape[0], *z.shape[1:]),
                                 z.dtype) for z in zero_outs]
        return [jax.device_put(a, shard) for a in concat_in + concat_zeros]

    def call(dev_args):
        outs = sharded(*dev_args)
        for o in outs:
            o.block_until_ready()
        return outs

    call.sharded = sharded
    result = (make_args, call)
    _CACHE[key] = result
    return result


def _prep_in_maps(x, w_qkv, w_out, skip=None):
    if skip is None:
        skip = DEFAULT_SKIP
    import ml_dtypes
    bf = ml_dtypes.bfloat16
    x = np.asarray(x, dtype=np.float32)
    w_qkv = np.asarray(w_qkv, dtype=np.float32)
    w_out = np.asarray(w_out, dtype=np.float32)
    msk = np.zeros((128, 256), dtype=np.float32)
    msk[:, 0:128] = np.triu(np.full((128, 128), NEG, dtype=np.float32), k=1)
    msk[:, 128:256] = np.eye(128, dtype=np.float32)
    msk = msk.astype(bf)
    in_maps = []
    xts = [np.ascontiguousarray(x[b].T).astype(bf) for b in range(B)]

    for core in range(8):
        b, g = divmod(core, 4)
        cl, ch = 256 * g, 256 * g + 256
        wqkv = np.ascontiguousarray(np.concatenate(
            [w_qkv[:, cl:ch], w_qkv[:, C + cl:C + ch],
             w_qkv[:, 2 * C + cl:2 * C + ch]], axis=1)).astype(bf)
        wo = np.ascontiguousarray(w_out[cl:ch, :]).astype(bf)
        in_maps.append({"xt": xts[b], "wqkv": wqkv, "wo": wo, "msk": msk})
    return in_maps


def kernel(x, w_qkv, w_out):
    run = _get_runner()
    in_maps = _prep_in_maps(x, w_qkv, w_out)
    results = run(in_maps)
    y = np.zeros((B, T, C), dtype=np.float32)
    for core in range(8):
        b = core // 4
        y[b] += results[core]["y"].astype(np.float32)
    return y


if __name__ == "__main__":
    rng = np.random.default_rng(0)
    x = rng.standard_normal((B, T, C)).astype(np.float32)
    w_qkv = (rng.standard_normal((C, 3 * C)) / np.sqrt(C)).astype(np.float32)
    w_out = (rng.standard_normal((C, C)) / np.sqrt(C)).astype(np.float32)
    y = kernel(x=x, w_qkv=w_qkv, w_out=w_out)
    print("kernel ran, y:", y.shape, y.dtype, float(np.abs(y).max()))

